# revision 1
# baseline (speedup 1.0000x reference)
"""Trainium2 Bass kernel for nn_DTransformer (sparse decay attention layer).

Sharding: 8 cores = 4 batches x 2 interleaved q-stripes.
  stripe 0 -> q-tiles {0,3,4,7}, stripe 1 -> q-tiles {1,2,5,6} (of 8 tiles
  of 128 rows).  Both stripes have equal causal work (18 k-tile units).
  Each core computes q/k/v projections for its batch (q only for its own
  rows), runs the decay-attention for all 16 heads on its q rows, applies
  the output projection + residual + LayerNorm, and writes its 512 rows.
Two SPMD programs (one per stripe, different static causal extents), run
concurrently on devices 0-3 / 4-7.
"""

import numpy as np

import concourse.bacc as bacc
import concourse.tile as tile
import concourse.bass as bass
from concourse import mybir
from concourse.masks import make_identity

P = 128
F32 = mybir.dt.float32
F32R = mybir.dt.float32r
BF16 = mybir.dt.bfloat16
AF = mybir.ActivationFunctionType
ALU = mybir.AluOpType
NEG = -1.0e30
LOG1EM5 = float(np.log(1e-5))

B, T, D, H = 4, 1024, 1024, 16
QTILES_A = [0, 3, 4, 7]
QTILES_B = [1, 2, 5, 6]


def r32(ap):
    return ap.bitcast(F32R)


def emit(tc, io, qtiles, T=T, D=D, H=H, reps=1, parts=('w', 'attn', 'out')):
    """Emit the per-core program.  io: dict name -> bass.AP (DRAM)."""
    nc = tc.nc
    dk = D // H
    DT = D // P          # d chunks (contraction)
    ET = D // P          # e tiles
    TT = T // P          # total k tiles
    NQ = len(qtiles)
    TQ = NQ * P
    TK = (max(qtiles) + 1) * P   # k extent needed
    HPT = P // dk                # heads per e-tile (2)
    FH = max(1, D // 512)        # 512-wide column chunks of D
    eps = 1e-5

    xq, xk, xv = io["xq"], io["xk"], io["xv"]
    wq, wv, wo = io["wq"], io["wv"], io["wo"]
    bq, bv, bo = io["bq"], io["bv"], io["bo"]
    gam, lng, lnb = io["gam"], io["lng"], io["lnb"]
    y = io["y"]

    from contextlib import ExitStack
    _stack = ExitStack()
    cpool = _stack.enter_context(tc.tile_pool(name="consts", bufs=1))
    # persistent big tensors
    ppool = _stack.enter_context(tc.tile_pool(name="persist", bufs=1))

    ident = cpool.tile([P, P], F32)
    make_identity(nc, ident)
    maskdiag = cpool.tile([P, P], F32)   # -1e30 on j >= q (strict causal)
    nc.gpsimd.memset(maskdiag, 0.0)
    nc.gpsimd.affine_select(
        out=maskdiag, in_=maskdiag, compare_op=ALU.is_ge, fill=NEG,
        base=-1, pattern=[[-1, P]], channel_multiplier=1,
    )  # keep where q - j - 1 >= 0, i.e. j < q ; else -1e30
    ones_stage = cpool.tile([1, 512], F32)
    nc.vector.memset(ones_stage, 1.0)
    ones128 = cpool.tile([1, P], F32R)
    nc.vector.tensor_copy(out=ones128, in_=ones_stage[:, :P])
    ones512 = cpool.tile([1, 512], F32R)
    nc.vector.tensor_copy(out=ones512, in_=ones_stage)

    bstage = cpool.tile([1, 3 * D], F32)
    nc.sync.dma_start(out=bstage[:, 0:D], in_=bq[None, :])
    nc.sync.dma_start(out=bstage[:, D:2 * D], in_=bv[None, :])
    nc.sync.dma_start(out=bstage[:, 2 * D:3 * D], in_=bo[None, :])
    bq_sb = cpool.tile([1, D], F32R)
    nc.vector.tensor_copy(out=bq_sb, in_=bstage[:, 0:D])
    bv_sb = cpool.tile([1, D], F32R)
    nc.vector.tensor_copy(out=bv_sb, in_=bstage[:, D:2 * D])
    bo_sb = cpool.tile([1, D], F32R)
    nc.vector.tensor_copy(out=bo_sb, in_=bstage[:, 2 * D:3 * D])
    lng_bc = cpool.tile([P, D], F32)
    nc.sync.dma_start(out=lng_bc, in_=bass.AP(tensor=lng.tensor, offset=lng.offset,
                                             ap=[[0, P]] + lng.ap))
    lnb_bc = cpool.tile([P, D], F32)
    nc.sync.dma_start(out=lnb_bc, in_=bass.AP(tensor=lnb.tensor, offset=lnb.offset,
                                              ap=[[0, P]] + lnb.ap))
    # -|gamma| broadcast on all partitions: (P, H)
    gneg = cpool.tile([P, H], F32)
    nc.sync.dma_start(out=gneg, in_=bass.AP(tensor=gam.tensor, offset=gam.offset,
                                            ap=[[0, P]] + gam.ap))
    nc.scalar.activation(out=gneg, in_=gneg, func=AF.Abs)
    nc.vector.tensor_scalar_mul(gneg, gneg, -1.0)
    eps_col = cpool.tile([P, 1], F32)
    nc.vector.memset(eps_col, eps)

    # persistent tensors
    kT = ppool.tile([P, ET, TK], F32R, tag="kT")
    qT = ppool.tile([P, ET, TQ], F32R, tag="qT")
    vb = ppool.tile([P, TT, D], BF16, tag="v")
    qnat = ppool.tile([P, NQ, D], F32, tag="qnat")

    def body(_i=None):
        if 'w' not in parts:
            # still touch inputs/output so the DMA structure stays comparable
            pass
        # ---------------- phase W: transposes + projections ----------------
        with tc.tile_pool(name="wtmp", bufs=2) as wtmp, \
             tc.tile_pool(name="wpsum", bufs=2, space="PSUM") as wpsum, \
             tc.tile_pool(name="xtb", bufs=1) as xtb, \
             tc.tile_pool(name="wtb", bufs=1) as wtb:

            def build_wT(wsrc, n_ec):
                """load weight (e,d) row-tiles, transpose -> (d,e) tiles"""
                wT = wtb.tile([P, DT, D], F32R, tag="wT")
                for ec in range(n_ec):
                    wrow = wtmp.tile([P, D], F32, tag="wrow")
                    nc.sync.dma_start(out=wrow, in_=wsrc[ec * P:(ec + 1) * P, :])
                    for dc in range(DT):
                        tp = wpsum.tile([P, P], F32, tag="tp")
                        nc.tensor.transpose(tp, wrow[:, dc * P:(dc + 1) * P], ident)
                        nc.vector.tensor_copy(out=wT[:, dc, ec * P:(ec + 1) * P], in_=tp)
                return wT

            def build_xT(xsrc, n_t):
                """load activation (t,d) tiles, transpose -> xT (d, t)"""
                xT = xtb.tile([P, DT, T], F32R, tag="xT")
                for tt in range(n_t):
                    xrow = wtmp.tile([P, D], F32, tag="xrow")
                    nc.sync.dma_start(out=xrow, in_=xsrc[tt * P:(tt + 1) * P, :])
                    for dc in range(DT):
                        tp = wpsum.tile([P, P], F32, tag="tp")
                        nc.tensor.transpose(tp, xrow[:, dc * P:(dc + 1) * P], ident)
                        nc.vector.tensor_copy(out=xT[:, dc, tt * P:(tt + 1) * P], in_=tp)
                return xT

            # values -> v (natural t,e) in bf16
            wvT = build_wT(wv, ET)
            xvT = build_xT(xv, TT)
            for tt in range(TT):
                for fh in range(FH):
                    w = min(512, D)
                    ps = wpsum.tile([P, 512], F32, tag="proj")
                    for dc in range(DT):
                        nc.tensor.matmul(ps[:, :w], r32(xvT[:, dc, tt * P:(tt + 1) * P]),
                                         r32(wvT[:, dc, fh * 512:fh * 512 + w]),
                                         start=(dc == 0), stop=False)
                    nc.tensor.matmul(ps[:, :w], r32(ones128),
                                     r32(bv_sb[:, fh * 512:fh * 512 + w]),
                                     start=False, stop=True)
                    nc.vector.tensor_copy(out=vb[:, tt, fh * 512:fh * 512 + w], in_=ps[:, :w])

            # key -> kT (e, t)
            wqT = build_wT(wq, ET)
            xkT = build_xT(xk, TK // P)
            for et in range(ET):
                for th in range(0, TK, 512):
                    w = min(512, TK - th)
                    ps = wpsum.tile([P, 512], F32, tag="proj")
                    for dc in range(DT):
                        nc.tensor.matmul(ps[:, :w], r32(wqT[:, dc, et * P:(et + 1) * P]),
                                         r32(xkT[:, dc, th:th + w]),
                                         start=(dc == 0), stop=False)
                    nc.tensor.matmul(ps[:, :w], r32(bq_sb[:, et * P:(et + 1) * P]),
                                     r32(ones512[:, :w]), start=False, stop=True)
                    nc.scalar.copy(out=kT[:, et, th:th + w], in_=ps[:, :w])

            # query rows -> qnat (residual) and qT (e,t), prescaled by 1/8
            for j in range(NQ):
                nc.sync.dma_start(out=qnat[:, j, :], in_=xq[j * P:(j + 1) * P, :])
            xqT = xtb.tile([P, DT, T], F32R, tag="xT")  # reuse buffer slot
            for j in range(NQ):
                for dc in range(DT):
                    tp = wpsum.tile([P, P], F32, tag="tp")
                    nc.tensor.transpose(tp, qnat[:, j, dc * P:(dc + 1) * P], ident)
                    nc.vector.tensor_copy(out=xqT[:, dc, j * P:(j + 1) * P], in_=tp)
            for et in range(ET):
                for th in range(0, TQ, 512):
                    w = min(512, TQ - th)
                    ps = wpsum.tile([P, 512], F32, tag="proj")
                    for dc in range(DT):
                        nc.tensor.matmul(ps[:, :w], r32(wqT[:, dc, et * P:(et + 1) * P]),
                                         r32(xqT[:, dc, th:th + w]),
                                         start=(dc == 0), stop=False)
                    nc.tensor.matmul(ps[:, :w], r32(bq_sb[:, et * P:(et + 1) * P]),
                                     r32(ones512[:, :w]), start=False, stop=True)
                    nc.scalar.mul(out=qT[:, et, th:th + w], in_=ps[:, :w],
                                  mul=1.0 / float(np.sqrt(dk)))

        if 'attn' not in parts:
            return
        # ---------------- late phase: woT/concT + attention + output ----------------
        with tc.tile_pool(name="late", bufs=1) as lpool:
            woT = lpool.tile([P, ET, D], F32R, tag="woT")
            concT = lpool.tile([P, ET, TQ], F32R, tag="concT")
            with tc.tile_pool(name="wo_tmp", bufs=2) as wo_tmp, \
                 tc.tile_pool(name="wo_psum", bufs=2, space="PSUM") as wo_psum:
                for ec in range(ET):
                    wrow = wo_tmp.tile([P, D], F32, tag="worow")
                    nc.sync.dma_start(out=wrow, in_=wo[ec * P:(ec + 1) * P, :])
                    for dc in range(DT):
                        tp = wo_psum.tile([P, P], F32, tag="wo_tp")
                        nc.tensor.transpose(tp, wrow[:, dc * P:(dc + 1) * P], ident)
                        nc.vector.tensor_copy(out=woT[:, dc, ec * P:(ec + 1) * P], in_=tp)
            self_attention(lpool, woT, concT)

    def self_attention(lpool, woT, concT):
        with tc.tile_pool(name="atmp", bufs=2) as atmp, \
             tc.tile_pool(name="amini", bufs=2) as amini, \
             tc.tile_pool(name="negp_pool", bufs=1) as negp_pool, \
             tc.tile_pool(name="spsum", bufs=2, space="PSUM") as spsum, \
             tc.tile_pool(name="tpsum", bufs=2, space="PSUM") as tpsum, \
             tc.tile_pool(name="vpsum", bufs=2, space="PSUM") as vpsum:

            for j, gi in enumerate(qtiles):
                Lk = (gi + 1) * P
                nkt = gi + 1
                # negpos_cl[qi, k] = min(k - (gi*P + qi), 0)
                negp = negp_pool.tile([P, TK], F32, tag="negp")
                nc.gpsimd.iota(negp[:, :Lk], pattern=[[1, Lk]], base=-(gi * P),
                               channel_multiplier=-1,
                               allow_small_or_imprecise_dtypes=True)
                nc.gpsimd.tensor_scalar_min(negp[:, :Lk], negp[:, :Lk], 0.0)

                for hp in range(H // 2):
                    S_pair = []
                    xp = atmp.tile([P, 2, TK], F32, tag="xpair")
                    Zp = amini.tile([P, 2], F32, tag="Z")
                    lnZ = amini.tile([P, 2], F32, tag="lnZ")
                    cums = []
                    for hh in range(2):
                        h = 2 * hp + hh
                        et, po = h // HPT, (h % HPT) * dk
                        S = spsum.tile([P, TK], F32, tag="S")
                        S_pair.append(S)
                        for c0 in range(0, Lk, 512):
                            w = min(512, Lk - c0)
                            nc.tensor.matmul(
                                S[:, c0:c0 + w],
                                r32(qT[po:po + dk, et, j * P:(j + 1) * P]),
                                r32(kT[po:po + dk, et, c0:c0 + w]),
                                start=True, stop=True)
                        # strict causal mask on the diagonal tile
                        nc.vector.tensor_tensor(out=S[:, gi * P:Lk], in0=S[:, gi * P:Lk],
                                                in1=maskdiag, op=ALU.add)
                        # softmax1 numerator + row sum (no max-sub needed;
                        # logits are pre-scaled and O(1))
                        e_ = atmp.tile([P, TK], F32, tag="ebuf")
                        nc.scalar.activation(out=e_[:, :Lk], in_=S[:, :Lk], func=AF.Exp)
                        cum = atmp.tile([P, TK], F32, tag="cum")
                        nc.vector.tensor_tensor_scan(
                            out=cum[:, :Lk], data0=e_[:, :Lk], data1=e_[:, :Lk],
                            initial=0.0, op0=ALU.add, op1=ALU.bypass)
                        # Z taken bit-exactly from the scan tail so cum <= Z
                        nc.vector.tensor_copy(out=Zp[:, hh:hh + 1],
                                              in_=cum[:, Lk - 1:Lk])
                        if gi == 0:
                            nc.vector.memset(Zp[0:1, hh:hh + 1], 1.0)
                        cums.append((e_, cum))
                        # x = (cum - Z) * negpos  (>= 0)
                        nc.vector.scalar_tensor_tensor(
                            out=xp[:, hh, :Lk], in0=cum[:, :Lk], scalar=Zp[:, hh:hh + 1],
                            in1=negp[:, :Lk], op0=ALU.subtract, op1=ALU.mult)
                    # ln(Z) for the pair
                    nc.scalar.activation(out=lnZ, in_=Zp, func=AF.Ln)
                    # batched: sqrt(x) = exp(0.5 ln x); ln(0) -> -inf -> 0
                    nc.scalar.activation(out=xp[:, :, :Lk], in_=xp[:, :, :Lk], func=AF.Ln)
                    nc.scalar.activation(out=xp[:, :, :Lk], in_=xp[:, :, :Lk], func=AF.Exp,
                                         scale=0.5)
                    # gz = -|gamma_h| / sqrt(Z)
                    rsq = amini.tile([P, 2], F32, tag="rsq")
                    nc.scalar.activation(out=rsq, in_=lnZ, func=AF.Exp, scale=-0.5)
                    gz = amini.tile([P, 2], F32, tag="gz")
                    nc.vector.tensor_tensor(out=gz, in0=rsq, in1=gneg[:, 2 * hp:2 * hp + 2],
                                            op=ALU.mult)
                    for hh in range(2):
                        # w = max(gz * sqrt(x), ln(1e-5))  [clip(e^w, 1e-5) folded]
                        nc.gpsimd.tensor_tensor(
                            out=xp[:, hh, :Lk], in0=xp[:, hh, :Lk],
                            in1=gz[:, hh:hh + 1].to_broadcast([P, Lk]), op=ALU.mult)
                        nc.gpsimd.tensor_scalar(
                            out=xp[:, hh, :Lk], in0=xp[:, hh, :Lk],
                            scalar1=LOG1EM5, scalar2=None, op0=ALU.max)
                    # f = exp(w), batched over the pair
                    nc.scalar.activation(out=xp[:, :, :Lk], in_=xp[:, :, :Lk], func=AF.Exp)

                    for hh in range(2):
                        h = 2 * hp + hh
                        S = S_pair[hh]
                        e_, cum = cums[hh]
                        # s2 = S * f   (reuse e_ buffer)
                        nc.vector.tensor_tensor(out=e_[:, :Lk], in0=S[:, :Lk],
                                                in1=xp[:, hh, :Lk], op=ALU.mult)
                        Z2 = amini.tile([P, 1], F32, tag="Z2")
                        nc.scalar.activation(out=e_[:, :Lk], in_=e_[:, :Lk], func=AF.Exp,
                                             accum_out=Z2)
                        m2 = amini.tile([P, 1], F32, tag="m2")
                        nc.vector.tensor_reduce(out=m2, in_=e_[:, :Lk],
                                                axis=mybir.AxisListType.X, op=ALU.max)
                        # c = min(Z2/m2, 5) / Z2
                        rz = amini.tile([P, 1], F32, tag="rz")
                        nc.vector.reciprocal(out=rz, in_=Z2)
                        rm = amini.tile([P, 1], F32, tag="rm")
                        nc.vector.reciprocal(out=rm, in_=m2)
                        sc = amini.tile([P, 1], F32, tag="sc")
                        nc.vector.tensor_tensor(out=sc, in0=Z2, in1=rm, op=ALU.mult)
                        nc.vector.tensor_scalar_min(sc, sc, 5.0)
                        cc = amini.tile([P, 1], F32, tag="cc")
                        nc.vector.tensor_tensor(out=cc, in0=sc, in1=rz, op=ALU.mult)
                        # p = e2 * c
                        nc.gpsimd.tensor_tensor(out=e_[:, :Lk], in0=e_[:, :Lk],
                                                in1=cc.to_broadcast([P, Lk]), op=ALU.mult)
                        if gi == 0:
                            nc.vector.memset(e_[0:1, :Lk], 0.0)
                        # transpose p 128x128 tiles -> bf16 pT
                        pT = atmp.tile([P, TK], BF16, tag="pT")
                        for g0 in range(0, nkt, 4):
                            gn = min(4, nkt - g0)
                            tp = tpsum.tile([P, 4 * P], F32, tag="ptp")
                            for gg in range(gn):
                                kt = g0 + gg
                                nc.tensor.transpose(
                                    tp[:, gg * P:(gg + 1) * P],
                                    e_[:, kt * P:(kt + 1) * P], ident)
                            nc.vector.tensor_copy(out=pT[:, g0 * P:(g0 + gn) * P],
                                                  in_=tp[:, :gn * P])
                        # out^T (dk, q) accumulated over k tiles
                        ov = vpsum.tile([dk, P], F32, tag="ov")
                        for kt in range(nkt):
                            nc.tensor.matmul(ov, vb[:, kt, h * dk:(h + 1) * dk],
                                             pT[:, kt * P:(kt + 1) * P],
                                             start=(kt == 0), stop=(kt == nkt - 1))
                        et, po = h // HPT, (h % HPT) * dk
                        nc.scalar.copy(out=concT[po:po + dk, et, j * P:(j + 1) * P], in_=ov)

        if 'out' not in parts:
            return
        # ---------------- output projection + residual + layernorm ----------------
        with tc.tile_pool(name="otmp", bufs=2) as otmp, \
             tc.tile_pool(name="omini", bufs=2) as omini, \
             tc.tile_pool(name="opsum", bufs=2, space="PSUM") as opsum:
            for j in range(NQ):
                xsb = otmp.tile([P, D], F32, tag="xsb")
                for fh in range(FH):
                    w = min(512, D - fh * 512)
                    ps = opsum.tile([P, 512], F32, tag="attn")
                    for et in range(ET):
                        nc.tensor.matmul(ps[:, :w], r32(concT[:, et, j * P:(j + 1) * P]),
                                         r32(woT[:, et, fh * 512:fh * 512 + w]),
                                         start=(et == 0), stop=False)
                    nc.tensor.matmul(ps[:, :w], r32(ones128),
                                     r32(bo_sb[:, fh * 512:fh * 512 + w]),
                                     start=False, stop=True)
                    # residual
                    nc.vector.tensor_tensor(out=xsb[:, fh * 512:fh * 512 + w],
                                            in0=ps[:, :w],
                                            in1=qnat[:, j, fh * 512:fh * 512 + w],
                                            op=ALU.add)
                # layernorm stats
                nst = D // 512 if D >= 512 else 1
                sw = D // nst
                stats = omini.tile([P, nst, 6], F32, tag="stats")
                for sg in range(nst):
                    nc.vector.bn_stats(out=stats[:, sg, :], in_=xsb[:, sg * sw:(sg + 1) * sw])
                mv = omini.tile([P, 2], F32, tag="mv")
                nc.vector.bn_aggr(out=mv, in_=stats)
                # rstd = exp(-0.5 ln(var + eps))
                rstd = omini.tile([P, 1], F32, tag="rstd")
                nc.scalar.activation(out=rstd, in_=mv[:, 1:2], func=AF.Ln, bias=eps_col)
                nc.scalar.activation(out=rstd, in_=rstd, func=AF.Exp, scale=-0.5)
                nmr = omini.tile([P, 1], F32, tag="nmr")
                nc.vector.tensor_tensor(out=nmr, in0=mv[:, 0:1], in1=rstd, op=ALU.mult)
                nc.vector.tensor_scalar_mul(nmr, nmr, -1.0)
                ysb = otmp.tile([P, D], F32, tag="ysb")
                nc.scalar.activation(out=ysb, in_=xsb, func=AF.Identity,
                                     bias=nmr, scale=rstd)
                nc.vector.tensor_tensor(out=ysb, in0=ysb, in1=lng_bc, op=ALU.mult)
                nc.vector.tensor_tensor(out=ysb, in0=ysb, in1=lnb_bc, op=ALU.add)
                nc.sync.dma_start(out=y[j * P:(j + 1) * P, :], in_=ysb)

    if reps == 1:
        body()
    else:
        with tc.For_i(0, reps, 1) as _i:
            body(_i)

    _stack.close()


# ------------------------------------------------------------------
# program build + host-side runner
# ------------------------------------------------------------------

def build_program(qtiles, T=T, D=D, H=H, reps=1, parts=('w', 'attn', 'out')):
    NQ = len(qtiles)
    nc = bacc.Bacc("TRN2", target_bir_lowering=False, debug=False, num_devices=4)
    io = {}
    io["xq"] = nc.dram_tensor("xq", [NQ * P, D], F32, kind="ExternalInput").ap()
    io["xk"] = nc.dram_tensor("xk", [T, D], F32, kind="ExternalInput").ap()
    io["xv"] = nc.dram_tensor("xv", [T, D], F32, kind="ExternalInput").ap()
    io["wq"] = nc.dram_tensor("wq", [D, D], F32, kind="ExternalInput").ap()
    io["wv"] = nc.dram_tensor("wv", [D, D], F32, kind="ExternalInput").ap()
    io["wo"] = nc.dram_tensor("wo", [D, D], F32, kind="ExternalInput").ap()
    io["bq"] = nc.dram_tensor("bq", [D], F32, kind="ExternalInput").ap()
    io["bv"] = nc.dram_tensor("bv", [D], F32, kind="ExternalInput").ap()
    io["bo"] = nc.dram_tensor("bo", [D], F32, kind="ExternalInput").ap()
    io["gam"] = nc.dram_tensor("gam", [H], F32, kind="ExternalInput").ap()
    io["lng"] = nc.dram_tensor("lng", [D], F32, kind="ExternalInput").ap()
    io["lnb"] = nc.dram_tensor("lnb", [D], F32, kind="ExternalInput").ap()
    io["y"] = nc.dram_tensor("y", [NQ * P, D], F32, kind="ExternalOutput").ap()
    with tile.TileContext(nc) as tc:
        emit(tc, io, qtiles, T=T, D=D, H=H, reps=reps, parts=parts)
    nc.compile()
    return nc


def make_in_maps(inputs, qtiles):
    """Per-core input dicts for one stripe (4 cores, batches 0..3)."""
    q = np.asarray(inputs["query"], np.float32)
    k = np.asarray(inputs["key"], np.float32)
    v = np.asarray(inputs["values"], np.float32)
    rows = np.concatenate([np.arange(g * P, (g + 1) * P) for g in qtiles])
    shared = {
        "wq": np.ascontiguousarray(inputs["Wq"], np.float32),
        "wv": np.ascontiguousarray(inputs["Wv"], np.float32),
        "wo": np.ascontiguousarray(inputs["Wo"], np.float32),
        "bq": np.ascontiguousarray(inputs["bq"], np.float32),
        "bv": np.ascontiguousarray(inputs["bv"], np.float32),
        "bo": np.ascontiguousarray(inputs["bo"], np.float32),
        "gam": np.ascontiguousarray(inputs["gammas"], np.float32),
        "lng": np.ascontiguousarray(inputs["ln_g"], np.float32),
        "lnb": np.ascontiguousarray(inputs["ln_b"], np.float32),
    }
    maps = []
    for b in range(B):
        m = dict(shared)
        m["xq"] = np.ascontiguousarray(q[b][rows])
        m["xk"] = np.ascontiguousarray(k[b])
        m["xv"] = np.ascontiguousarray(v[b])
        maps.append(m)
    return maps


class _Runner:
    """Runs the two stripe programs concurrently on devices 0-3 / 4-7."""

    def __init__(self, reps=1):
        self.nc_a = build_program(QTILES_A, reps=reps)
        self.nc_b = build_program(QTILES_B, reps=reps)
        self._fns = None

    def _make_fn(self, nc, devices):
        import jax
        from jax.sharding import Mesh, PartitionSpec
        from jax.experimental.shard_map import shard_map
        from concourse import bass2jax
        from concourse.bass2jax import _bass_exec_p, partition_id_tensor

        bass2jax.install_neuronx_cc_hook()
        partition_name = (nc.partition_id_tensor.name
                          if nc.partition_id_tensor else None)
        in_names, out_names, out_avals, zero_outs = [], [], [], []
        for alloc in nc.m.functions[0].allocations:
            if not isinstance(alloc, mybir.MemoryLocationSet):
                continue
            name = alloc.memorylocations[0].name
            if alloc.kind == "ExternalInput":
                if name != partition_name:
                    in_names.append(name)
            elif alloc.kind == "ExternalOutput":
                shape = tuple(alloc.tensor_shape)
                dtype = mybir.dt.np(alloc.dtype)
                out_names.append(name)
                out_avals.append(jax.core.ShapedArray(shape, dtype))
                zero_outs.append(np.zeros(shape, dtype))
        n_params = len(in_names)
        all_in = list(in_names) + list(out_names)
        if partition_name is not None:
            all_in.append(partition_name)

        def _body(*args):
            operands = list(args)
            if partition_name is not None:
                operands.append(partition_id_tensor())
            outs = _bass_exec_p.bind(
                *operands, out_avals=tuple(out_avals), in_names=tuple(all_in),
                out_names=tuple(out_names), lowering_input_output_aliases=(),
                sim_require_finite=True, sim_require_nnan=True, nc=nc)
            return tuple(outs)

        mesh = Mesh(np.asarray(devices), ("core",))
        n = n_params + len(out_names)
        fn = jax.jit(shard_map(_body, mesh=mesh,
                               in_specs=(PartitionSpec("core"),) * n,
                               out_specs=(PartitionSpec("core"),) * len(out_names),
                               check_rep=False),
                     keep_unused=True)
        return fn, in_names, out_names, zero_outs

    def fns(self):
        if self._fns is None:
            import jax
            devs = jax.devices()
            self._fns = (self._make_fn(self.nc_a, devs[0:4]),
                         self._make_fn(self.nc_b, devs[4:8]))
        return self._fns

    def _concat_args(self, spec, in_maps):
        fn, in_names, out_names, zero_outs = spec
        args = [np.concatenate([np.asarray(m[nm]) for m in in_maps], axis=0)
                for nm in in_names]
        args += [np.zeros((4 * z.shape[0], *z.shape[1:]), z.dtype) for z in zero_outs]
        return args

    def run(self, inputs):
        import jax
        spec_a, spec_b = self.fns()
        maps_a = make_in_maps(inputs, QTILES_A)
        maps_b = make_in_maps(inputs, QTILES_B)
        oa = spec_a[0](*self._concat_args(spec_a, maps_a))
        ob = spec_b[0](*self._concat_args(spec_b, maps_b))
        jax.block_until_ready((oa, ob))
        ya = np.asarray(oa[0]).reshape(4, len(QTILES_A) * P, D)
        yb = np.asarray(ob[0]).reshape(4, len(QTILES_B) * P, D)
        out = np.empty((B, T, D), np.float32)
        for b in range(B):
            for jj, g in enumerate(QTILES_A):
                out[b, g * P:(g + 1) * P] = ya[b, jj * P:(jj + 1) * P]
            for jj, g in enumerate(QTILES_B):
                out[b, g * P:(g + 1) * P] = yb[b, jj * P:(jj + 1) * P]
        return out


_runner = None


def kernel(**inputs) -> np.ndarray:
    global _runner
    if _runner is None:
        _runner = _Runner(reps=1)
    return _runner.run(inputs)



# revision 14
# speedup vs baseline: 5519.4327x; 5519.4327x over previous
"""Trainium2 Bass kernel for nn_DTransformer (sparse decay attention layer).

Single SPMD program on 8 cores: core c -> (stripe = c//4, batch = c%4).
Stripe A owns q-tiles {0,3,4,7}, stripe B {1,2,5,6} (of 8 tiles of 128
rows).  Both stripes process their j-th tile with a PADDED causal extent
EXT[j] = (2,4,6,8) k-tiles so the instruction stream is identical; the
true causal structure is injected via per-core input data (additive mask
tiles, iota offsets, zfix columns).

Math per (q-tile, head), S = (q@k^T)/sqrt(dk) with strict-causal mask:
  e    = exp(S)                       [ACT, accum -> Z]
  suf  = reversed-exclusive-cumsum(e) [DVE scan, neg-stride APs]
  sq   = suf^0.5 * sqrt(i-k)          [DVE stt pow+mult; sqrt((Z-cum)*pos)
                                       factorized]
  f    = exp(gz * sq), gz = -|gamma|/sqrt(Z)   [ACT, per-row scale]
  e2   = e^f = exp(S*f)               [DVE stt pow, accum -> Z2, bf16]
  m2   = rowmax(e2)                   [DVE tensor_mask_reduce]
  p    = e2 * min(Z2/m2,5)/Z2         [DVE tensor_scalar, bf16]
  out += p @ v                        [PE: transpose p + matmul]
Projections / output proj / residual / LayerNorm standard; weights and
activations are fed pre-transposed and bf16-cast from the host.
"""

import numpy as np

import concourse.bacc as bacc
import concourse.tile as tile
import concourse.bass as bass
from concourse import mybir
from concourse.masks import make_identity

P = 128
F32 = mybir.dt.float32
BF16 = mybir.dt.bfloat16
AF = mybir.ActivationFunctionType
ALU = mybir.AluOpType
NEG = -1.0e30

B, T, D, H = 4, 1024, 1024, 16
dk = D // H          # 64
ET = D // P          # 8 e-tiles
DT = D // P          # 8 contraction tiles
TT = T // P          # 8 t-tiles
NQ = 4               # q-tiles per core
TQ = NQ * P          # 512 q rows per core
EXT = [2, 4, 6, 8]   # padded k-extents (tiles) per q-slot, both stripes
QT_A = [0, 3, 4, 7]
QT_B = [1, 2, 5, 6]
EPS = 1e-5


def emit(tc, io):
    nc = tc.nc
    from contextlib import ExitStack
    st = ExitStack()

    cpool = st.enter_context(tc.tile_pool(name="consts", bufs=1))
    ppool = st.enter_context(tc.tile_pool(name="persist", bufs=1))

    # ---------------- constants ----------------
    ident = cpool.tile([P, P], F32)
    make_identity(nc, ident)
    ident_bf = cpool.tile([P, P], BF16)
    nc.vector.tensor_copy(out=ident_bf, in_=ident)

    ones1_bf = cpool.tile([1, P], BF16)
    nc.vector.memset(ones1_bf, 1.0)

    # biases per-partition per-etile: bq_pe[p, et] = bq[et*128+p]
    bq_pe = cpool.tile([P, ET], F32)
    nc.sync.dma_start(out=bq_pe, in_=bass.AP(
        tensor=io["bq"].tensor, offset=io["bq"].offset, ap=[[1, P], [P, ET]]))
    bq8 = cpool.tile([P, ET], F32)
    nc.vector.tensor_scalar_mul(bq8, bq_pe, 0.125)

    stage = cpool.tile([1, 2 * D], F32)
    nc.sync.dma_start(out=stage[:, 0:D], in_=io["bv"][None, :])
    nc.sync.dma_start(out=stage[:, D:2 * D], in_=io["bo"][None, :])
    bv_bf = cpool.tile([1, D], BF16)
    nc.vector.tensor_copy(out=bv_bf, in_=stage[:, 0:D])
    bo_bf = cpool.tile([1, D], BF16)
    nc.vector.tensor_copy(out=bo_bf, in_=stage[:, D:2 * D])

    def bcast(src, cols, tag):
        t = cpool.tile([P, cols], F32, tag=tag)
        nc.sync.dma_start(out=t, in_=bass.AP(
            tensor=src.tensor, offset=src.offset, ap=[[0, P]] + src.ap))
        return t

    lng_bc = bcast(io["lng"], D, "lng_bc")
    lnb_bc = bcast(io["lnb"], D, "lnb_bc")
    gam_bc = bcast(io["gam"], H, "gam_bc")
    gneg = cpool.tile([P, H], F32)
    nc.scalar.activation(out=gneg, in_=gam_bc, func=AF.Abs)
    nc.vector.tensor_scalar_mul(gneg, gneg, -1.0)

    iota = cpool.tile([P, T], F32)        # k - qi
    nc.sync.dma_start(out=iota, in_=io["iota"])
    gcol = cpool.tile([P, NQ], F32)       # -gi*128 per slot
    nc.sync.dma_start(out=gcol, in_=io["gcol"])
    zfix = cpool.tile([P, NQ], F32)
    nc.sync.dma_start(out=zfix, in_=io["zfix"])
    maskme = cpool.tile([P, NQ * 2 * P], BF16)   # additive mask, last 2 tiles/slot
    mstage = cpool.tile([P, NQ * 2 * P], F32)
    nc.sync.dma_start(out=mstage, in_=io["maskme"])
    nc.vector.tensor_copy(out=maskme, in_=mstage)

    eps_col = cpool.tile([P, 1], F32)
    nc.vector.memset(eps_col, EPS)

    # ---------------- persistent activations ----------------
    kT = ppool.tile([P, ET, T], BF16, tag="kT")       # (e, t) per e-tile
    qT = ppool.tile([P, ET, TQ], BF16, tag="qT")      # (e, q) prescaled 1/8
    vb = ppool.tile([P, TT, D], BF16, tag="vb")       # (t, e) natural
    concT = ppool.tile([P, ET, TQ], BF16, tag="concT")

    # ---------------- projections ----------------
    with tc.tile_pool(name="wx", bufs=1) as wx, \
         tc.tile_pool(name="ppsum", bufs=1, space="PSUM") as pp, \
         tc.tile_pool(name="vpsum", bufs=2, space="PSUM") as vp:
        wqT_sb = wx.tile([P, DT, D], BF16, tag="wqT")
        wvT_sb = wx.tile([P, DT, D], BF16, tag="wvT")
        xkT_sb = wx.tile([P, DT, T], BF16, tag="xkT")
        xvT_sb = wx.tile([P, DT, T], BF16, tag="xvT")
        xqT_sb = wx.tile([P, DT, TQ], BF16, tag="xqT")
        for dc in range(DT):
            r = slice(dc * P, (dc + 1) * P)
            nc.sync.dma_start(out=wqT_sb[:, dc, :], in_=io["wqT"][r, :])
            nc.sync.dma_start(out=xkT_sb[:, dc, :], in_=io["xkT"][r, :])
            nc.sync.dma_start(out=xqT_sb[:, dc, :], in_=io["xqT"][r, :])
            nc.sync.dma_start(out=wvT_sb[:, dc, :], in_=io["wvT"][r, :])
            nc.sync.dma_start(out=xvT_sb[:, dc, :], in_=io["xvT"][r, :])

        # k/q projections, et-major; shared stationary weight per (et, dc)
        for et in range(ET):
            es = slice(et * P, (et + 1) * P)
            kq = pp.tile([P, T], F32, tag="kq")
            qq = pp.tile([P, TQ], F32, tag="qq")
            for dc in range(DT):
                lw = wqT_sb[:, dc, es]
                nc.tensor.matmul(kq[:, 0:512], lw, xkT_sb[:, dc, 0:512],
                                 start=(dc == 0), stop=(dc == DT - 1))
                nc.tensor.matmul(kq[:, 512:1024], lw, xkT_sb[:, dc, 512:1024],
                                 start=(dc == 0), stop=(dc == DT - 1))
                nc.tensor.matmul(qq, lw, xqT_sb[:, dc, :],
                                 start=(dc == 0), stop=(dc == DT - 1))
            nc.scalar.activation(out=kT[:, et, :], in_=kq, func=AF.Identity,
                                 bias=bq_pe[:, et:et + 1])
            nc.scalar.activation(out=qT[:, et, :], in_=qq, func=AF.Identity,
                                 bias=bq8[:, et:et + 1], scale=0.125)

        # v projection, tt-major
        for tt in range(TT):
            ts_ = slice(tt * P, (tt + 1) * P)
            for fh in range(2):
                fs = slice(fh * 512, (fh + 1) * 512)
                vv = vp.tile([P, 512], F32, tag="vv")
                for dc in range(DT):
                    nc.tensor.matmul(vv, xvT_sb[:, dc, ts_], wvT_sb[:, dc, fs],
                                     start=(dc == 0), stop=False)
                nc.tensor.matmul(vv, ones1_bf, bv_bf[:, fs],
                                 start=False, stop=True)
                nc.scalar.activation(out=vb[:, tt, fs], in_=vv, func=AF.Copy)

    # ---------------- late weights ----------------
    lpool = st.enter_context(tc.tile_pool(name="late", bufs=1))
    woT_sb = lpool.tile([P, DT, D], BF16, tag="woT")
    qnat = lpool.tile([P, NQ, D], F32, tag="qnat")
    for dc in range(DT):
        nc.sync.dma_start(out=woT_sb[:, dc, :],
                          in_=io["woT"][dc * P:(dc + 1) * P, :])
    for j in range(NQ):
        nc.sync.dma_start(out=qnat[:, j, :], in_=io["xq"][j * P:(j + 1) * P, :])

    # ---------------- attention + output ----------------
    with tc.tile_pool(name="anp", bufs=2) as anp, \
         tc.tile_pool(name="abig", bufs=3) as abig, \
         tc.tile_pool(name="asm", bufs=2) as asm, \
         tc.tile_pool(name="spsum", bufs=2, space="PSUM") as sp, \
         tc.tile_pool(name="tpsum", bufs=1, space="PSUM") as tp_pool, \
         tc.tile_pool(name="vpsum2", bufs=2, space="PSUM") as pvp, \
         tc.tile_pool(name="opsum", bufs=1, space="PSUM") as op_pool, \
         tc.tile_pool(name="otmp", bufs=2) as otmp, \
         tc.tile_pool(name="omini", bufs=2) as omini:

        for j in range(NQ):
            nkt = EXT[j]
            Lk = nkt * P
            qs = slice(j * P, (j + 1) * P)

            negp = anp.tile([P, T], F32, tag="negp")
            nc.vector.tensor_scalar(out=negp[:, :Lk], in0=iota[:, :Lk],
                                    scalar1=gcol[:, j:j + 1], scalar2=0.0,
                                    op0=ALU.add, op1=ALU.min)
            # negp broadcast over a head pair: [P, 2, Lk] with 0-stride
            negp2 = bass.AP(tensor=negp.tensor, offset=negp.offset,
                            ap=[negp.ap[0], [0, 2], [1, Lk]])
            lkcol = anp.tile([P, 1], F32, tag="lkcol")
            nc.vector.memset(lkcol, float(Lk))

            for hp in range(H // 2):
                h0 = 2 * hp
                et = hp  # = h0 // 2
                Zp = asm.tile([P, 2], F32, tag="Zp")
                gz = asm.tile([P, 2], F32, tag="gz")
                Z2p = asm.tile([P, 2], F32, tag="Z2p")
                m2p = asm.tile([P, 2], F32, tag="m2p")
                Ss, e2s = [], []
                x_ = abig.tile([P, 2, T], F32, tag="x")
                # --- stage A per head: scores, exp(e) for the suffix scan ---
                for hh in range(2):
                    po = hh * dk
                    S = sp.tile([P, T], F32, tag="S")
                    Ss.append(S)
                    c0 = 0
                    while c0 < Lk - 256:
                        w = min(512, Lk - 256 - c0)
                        nc.tensor.matmul(S[:, c0:c0 + w],
                                         qT[po:po + dk, et, qs],
                                         kT[po:po + dk, et, c0:c0 + w],
                                         start=True, stop=True)
                        c0 += w
                    nc.tensor.matmul(S[:, Lk - 256:Lk],
                                     qT[po:po + dk, et, qs],
                                     kT[po:po + dk, et, Lk - 256:Lk],
                                     start=True, stop=False)
                    nc.tensor.matmul(S[:, Lk - 256:Lk], ident_bf,
                                     maskme[:, j * 256:(j + 1) * 256],
                                     start=False, stop=True)
                    e_ = abig.tile([P, T], F32, tag="e")
                    nc.scalar.activation(out=e_[:, :Lk], in_=S[:, :Lk],
                                         func=AF.Exp,
                                         accum_out=Zp[:, hh:hh + 1])
                    nc.vector.memset(x_[:, hh, Lk - 1:Lk], 0.0)
                    nc.vector.tensor_tensor_scan(
                        out=x_[:, hh, Lk - 2::-1], data0=e_[:, Lk - 1:0:-1],
                        data1=e_[:, Lk - 1:0:-1], initial=0.0,
                        op0=ALU.add, op1=ALU.bypass)
                # --- pair: x2 = suffix * (i-k); sqrt via ln/exp (same table
                # set as Exp); gz = -|gamma| / sqrt(Z + zfix) ---
                x2 = bass.AP(tensor=x_.tensor, offset=x_.offset,
                             ap=[x_.ap[0], [T, 2], [1, Lk]])
                nc.vector.scalar_tensor_tensor(
                    out=x2, in0=x2, scalar=-1.0,
                    in1=negp2, op0=ALU.mult, op1=ALU.mult)
                nc.scalar.activation(out=x2, in_=x2, func=AF.Ln)
                nc.scalar.activation(out=x2, in_=x2, func=AF.Exp, scale=0.5)
                nc.vector.tensor_tensor(out=Zp, in0=Zp,
                                        in1=zfix[:, j:j + 1].to_broadcast([P, 2]),
                                        op=ALU.add)
                nc.scalar.activation(out=gz, in_=Zp, func=AF.Ln)
                nc.scalar.activation(out=gz, in_=gz, func=AF.Exp, scale=-0.5)
                nc.vector.tensor_tensor(out=gz, in0=gz,
                                        in1=gneg[:, h0:h0 + 2], op=ALU.mult)
                # --- stage B per head: f, S2 = S*f (psum), e2, rowmax ---
                for hh in range(2):
                    S = Ss[hh]
                    f_ = abig.tile([P, T], F32, tag="f")
                    nc.scalar.activation(out=f_[:, :Lk], in_=x_[:, hh, :Lk],
                                         func=AF.Exp, scale=gz[:, hh:hh + 1])
                    nc.vector.scalar_tensor_tensor(
                        out=S[:, :Lk], in0=f_[:, :Lk], scalar=1.0,
                        in1=S[:, :Lk], op0=ALU.mult, op1=ALU.mult)
                    e2 = abig.tile([P, T], BF16, tag="e2")
                    e2s.append(e2)
                    nc.scalar.activation(out=e2[:, :Lk], in_=S[:, :Lk],
                                         func=AF.Exp,
                                         accum_out=Z2p[:, hh:hh + 1])
                    nc.vector.tensor_reduce(
                        out=m2p[:, hh:hh + 1], in_=e2[:, :Lk],
                        axis=mybir.AxisListType.X, op=ALU.max)
                # --- pair tiny: cc = min(Z2'/m2, 5) / Z2' ---
                nc.vector.tensor_tensor(out=Z2p, in0=Z2p,
                                        in1=zfix[:, j:j + 1].to_broadcast([P, 2]),
                                        op=ALU.add)
                nc.vector.tensor_tensor(out=m2p, in0=m2p,
                                        in1=zfix[:, j:j + 1].to_broadcast([P, 2]),
                                        op=ALU.add)
                cc = asm.tile([P, 2], F32, tag="cc")
                rz2 = asm.tile([P, 2], F32, tag="rz2")
                nc.vector.reciprocal(out=rz2, in_=Z2p)
                rm2 = asm.tile([P, 2], F32, tag="rm2")
                nc.vector.reciprocal(out=rm2, in_=m2p)
                nc.vector.tensor_tensor(out=cc, in0=Z2p, in1=rm2, op=ALU.mult)
                nc.vector.tensor_scalar_min(cc, cc, 5.0)
                nc.vector.tensor_tensor(out=cc, in0=cc, in1=rz2, op=ALU.mult)
                # --- stage C per head: rescale, transpose, PV ---
                pv = pvp.tile([dk, 2 * P], F32, tag="pv")
                for hh in range(2):
                    h = h0 + hh
                    e2 = e2s[hh]
                    p_ = asm.tile([P, T], BF16, tag="p")
                    nc.vector.tensor_scalar_mul(p_[:, :Lk], e2[:, :Lk],
                                                cc[:, hh:hh + 1])
                    pT = asm.tile([P, T], BF16, tag="pT")
                    for g0 in range(0, nkt, 4):
                        gn = min(4, nkt - g0)
                        tp = tp_pool.tile([P, 4 * P], BF16, tag="tp")
                        for gg in range(gn):
                            kt = g0 + gg
                            nc.tensor.transpose(tp[:, gg * P:(gg + 1) * P],
                                                p_[:, kt * P:(kt + 1) * P],
                                                ident_bf)
                        nc.scalar.activation(out=pT[:, g0 * P:(g0 + gn) * P],
                                             in_=tp[:, :gn * P], func=AF.Copy)
                    for kt in range(nkt):
                        nc.tensor.matmul(pv[:, hh * P:(hh + 1) * P],
                                         vb[:, kt, h * dk:(h + 1) * dk],
                                         pT[:, kt * P:(kt + 1) * P],
                                         start=(kt == 0), stop=(kt == nkt - 1))
                for hh in range(2):
                    nc.scalar.activation(
                        out=concT[hh * dk:(hh + 1) * dk, et, qs],
                        in_=pv[:, hh * P:(hh + 1) * P], func=AF.Copy)

            # ---- output projection + residual + layernorm for q-tile j ----
            xsb = otmp.tile([P, D], F32, tag="xsb")
            for fh in range(2):
                fs = slice(fh * 512, (fh + 1) * 512)
                ps = op_pool.tile([P, 512], F32, tag="op")
                for et2 in range(ET):
                    nc.tensor.matmul(ps, concT[:, et2, qs],
                                     woT_sb[:, et2, fs],
                                     start=(et2 == 0), stop=False)
                nc.tensor.matmul(ps, ones1_bf, bo_bf[:, fs],
                                 start=False, stop=True)
                nc.vector.tensor_tensor(out=xsb[:, fs], in0=ps,
                                        in1=qnat[:, j, fs], op=ALU.add)
            stats = omini.tile([P, 2, 6], F32, tag="stats")
            for sg in range(2):
                nc.vector.bn_stats(out=stats[:, sg, :],
                                   in_=xsb[:, sg * 512:(sg + 1) * 512])
            mv = omini.tile([P, 2], F32, tag="mv")
            nc.vector.bn_aggr(out=mv, in_=stats)
            rstd = omini.tile([P, 1], F32, tag="rstd")
            nc.scalar.activation(out=rstd, in_=mv[:, 1:2], func=AF.Ln,
                                 bias=eps_col)
            nc.scalar.activation(out=rstd, in_=rstd, func=AF.Exp, scale=-0.5)
            nmr = omini.tile([P, 1], F32, tag="nmr")
            nc.vector.scalar_tensor_tensor(out=nmr, in0=mv[:, 0:1], scalar=-1.0,
                                           in1=rstd, op0=ALU.mult, op1=ALU.mult)
            ysb = otmp.tile([P, D], F32, tag="ysb")
            nc.scalar.activation(out=ysb, in_=xsb, func=AF.Identity,
                                 bias=nmr, scale=rstd)
            nc.vector.tensor_tensor(out=ysb, in0=ysb, in1=lng_bc, op=ALU.mult)
            nc.vector.tensor_tensor(out=ysb, in0=ysb, in1=lnb_bc, op=ALU.add)
            nc.sync.dma_start(out=io["y"][qs, :], in_=ysb)

    st.close()


# ------------------------------------------------------------------
# program build + host-side runner
# ------------------------------------------------------------------

def build_program():
    nc = bacc.Bacc("TRN2", target_bir_lowering=False, debug=False,
                   num_devices=8)
    io = {}

    def inp(name, shape, dt=F32):
        io[name] = nc.dram_tensor(name, shape, dt, kind="ExternalInput").ap()

    inp("wqT", [D, D], BF16)
    inp("wvT", [D, D], BF16)
    inp("woT", [D, D], BF16)
    inp("xkT", [D, T], BF16)
    inp("xvT", [D, T], BF16)
    inp("xqT", [D, TQ], BF16)
    inp("xq", [TQ, D])
    inp("bq", [D])
    inp("bv", [D])
    inp("bo", [D])
    inp("gam", [H])
    inp("lng", [D])
    inp("lnb", [D])
    inp("iota", [P, T])
    inp("gcol", [P, NQ])
    inp("zfix", [P, NQ])
    inp("maskme", [P, NQ * 2 * P])
    io["y"] = nc.dram_tensor("y", [TQ, D], F32, kind="ExternalOutput").ap()
    with tile.TileContext(nc) as tc:
        emit(tc, io)
    nc.compile()
    return nc


def make_in_maps(inputs):
    import ml_dtypes
    bf = ml_dtypes.bfloat16
    q = np.asarray(inputs["query"], np.float32)
    k = np.asarray(inputs["key"], np.float32)
    v = np.asarray(inputs["values"], np.float32)
    wqT = np.ascontiguousarray(np.asarray(inputs["Wq"], np.float32).T).astype(bf)
    wvT = np.ascontiguousarray(np.asarray(inputs["Wv"], np.float32).T).astype(bf)
    woT = np.ascontiguousarray(np.asarray(inputs["Wo"], np.float32).T).astype(bf)
    small = {
        "bq": np.ascontiguousarray(inputs["bq"], np.float32),
        "bv": np.ascontiguousarray(inputs["bv"], np.float32),
        "bo": np.ascontiguousarray(inputs["bo"], np.float32),
        "gam": np.ascontiguousarray(inputs["gammas"], np.float32),
        "lng": np.ascontiguousarray(inputs["ln_g"], np.float32),
        "lnb": np.ascontiguousarray(inputs["ln_b"], np.float32),
    }
    iota = (np.arange(T)[None, :] - np.arange(P)[:, None]).astype(np.float32)

    stripe_data = []
    for qtiles in (QT_A, QT_B):
        rows = np.concatenate([np.arange(g * P, (g + 1) * P) for g in qtiles])
        gcol = np.zeros((P, NQ), np.float32)
        zfix = np.zeros((P, NQ), np.float32)
        maskme = np.zeros((P, NQ, 2, P), np.float32)
        for jj, gi in enumerate(qtiles):
            gcol[:, jj] = -float(gi * P)
            if gi == 0:
                zfix[0, jj] = 1.0
            i_glob = gi * P + np.arange(P)[:, None]
            for tt in range(2):
                tpos = EXT[jj] - 2 + tt
                kk = tpos * P + np.arange(P)[None, :]
                maskme[:, jj, tt, :] = np.where(kk >= i_glob, NEG, 0.0)
        stripe_data.append(dict(
            rows=rows, gcol=gcol, zfix=zfix,
            maskme=maskme.reshape(P, NQ * 2 * P)))

    maps = []
    for c in range(8):
        sd = stripe_data[c // 4]
        b = c % 4
        rows = sd["rows"]
        m = dict(small)
        m["wqT"], m["wvT"], m["woT"] = wqT, wvT, woT
        m["xkT"] = np.ascontiguousarray(k[b].T).astype(bf)
        m["xvT"] = np.ascontiguousarray(v[b].T).astype(bf)
        m["xqT"] = np.ascontiguousarray(q[b].T[:, rows]).astype(bf)
        m["xq"] = np.ascontiguousarray(q[b][rows])
        m["iota"] = iota
        m["gcol"] = sd["gcol"]
        m["zfix"] = sd["zfix"]
        m["maskme"] = sd["maskme"]
        maps.append(m)
    return maps


class _Runner:
    def __init__(self):
        self.nc = build_program()
        self._fn = None

    def _make_fn(self, nc, devices):
        import jax
        from jax.sharding import Mesh, PartitionSpec
        from jax.experimental.shard_map import shard_map
        from concourse import bass2jax
        from concourse.bass2jax import _bass_exec_p, partition_id_tensor

        bass2jax.install_neuronx_cc_hook()
        partition_name = (nc.partition_id_tensor.name
                          if nc.partition_id_tensor else None)
        in_names, out_names, out_avals, zero_outs = [], [], [], []
        for alloc in nc.m.functions[0].allocations:
            if not isinstance(alloc, mybir.MemoryLocationSet):
                continue
            name = alloc.memorylocations[0].name
            if alloc.kind == "ExternalInput":
                if name != partition_name:
                    in_names.append(name)
            elif alloc.kind == "ExternalOutput":
                shape = tuple(alloc.tensor_shape)
                dtype = mybir.dt.np(alloc.dtype)
                out_names.append(name)
                out_avals.append(jax.core.ShapedArray(shape, dtype))
                zero_outs.append(np.zeros(shape, dtype))
        n_params = len(in_names)
        all_in = list(in_names) + list(out_names)
        if partition_name is not None:
            all_in.append(partition_name)

        def _body(*args):
            operands = list(args)
            if partition_name is not None:
                operands.append(partition_id_tensor())
            outs = _bass_exec_p.bind(
                *operands, out_avals=tuple(out_avals), in_names=tuple(all_in),
                out_names=tuple(out_names), lowering_input_output_aliases=(),
                sim_require_finite=True, sim_require_nnan=True, nc=nc)
            return tuple(outs)

        mesh = Mesh(np.asarray(devices), ("core",))
        n = n_params + len(out_names)
        fn = jax.jit(shard_map(_body, mesh=mesh,
                               in_specs=(PartitionSpec("core"),) * n,
                               out_specs=(PartitionSpec("core"),) * len(out_names),
                               check_rep=False),
                     keep_unused=True)
        return fn, in_names, out_names, zero_outs

    def fn(self):
        if self._fn is None:
            import jax
            self._fn = self._make_fn(self.nc, jax.devices()[:8])
        return self._fn

    def run(self, inputs):
        import jax
        fn, in_names, out_names, zero_outs = self.fn()
        maps = make_in_maps(inputs)
        args = [np.concatenate([np.asarray(m[nm]) for m in maps], axis=0)
                for nm in in_names]
        args += [np.zeros((8 * z.shape[0], *z.shape[1:]), z.dtype)
                 for z in zero_outs]
        outs = fn(*args)
        jax.block_until_ready(outs)
        y = np.asarray(outs[0]).reshape(8, TQ, D)
        out = np.empty((B, T, D), np.float32)
        for c in range(8):
            qtiles = (QT_A, QT_B)[c // 4]
            b = c % 4
            for jj, g in enumerate(qtiles):
                out[b, g * P:(g + 1) * P] = y[c, jj * P:(jj + 1) * P]
        return out


_runner = None


def kernel(**inputs) -> np.ndarray:
    global _runner
    if _runner is None:
        _runner = _Runner()
    return _runner.run(inputs)


# revision 16
# speedup vs baseline: 7148.6787x; 1.2952x over previous
"""Trainium2 Bass kernel for nn_DTransformer (sparse decay attention layer).

Single SPMD program on 8 cores: core c -> (stripe = c//4, batch = c%4).
Stripe A owns q-tiles {0,3,4,7}, stripe B {1,2,5,6} (of 8 tiles of 128
rows).  Both stripes process their j-th tile with a PADDED causal extent
EXT[j] = (2,4,6,8) k-tiles so the instruction stream is identical; the
true causal structure is injected via per-core input data (additive mask
tiles, iota offsets, zfix columns).

Math per (q-tile, head), S = (q@k^T)/sqrt(dk) with strict-causal mask:
  e    = exp(S)                       [ACT, accum -> Z]
  suf  = reversed-exclusive-cumsum(e) [DVE scan, neg-stride APs]
  sq   = suf^0.5 * sqrt(i-k)          [DVE stt pow+mult; sqrt((Z-cum)*pos)
                                       factorized]
  f    = exp(gz * sq), gz = -|gamma|/sqrt(Z)   [ACT, per-row scale]
  e2   = e^f = exp(S*f)               [DVE stt pow, accum -> Z2, bf16]
  m2   = rowmax(e2)                   [DVE tensor_mask_reduce]
  p    = e2 * min(Z2/m2,5)/Z2         [DVE tensor_scalar, bf16]
  out += p @ v                        [PE: transpose p + matmul]
Projections / output proj / residual / LayerNorm standard; weights and
activations are fed pre-transposed and bf16-cast from the host.
"""

import numpy as np

import concourse.bacc as bacc
import concourse.tile as tile
import concourse.bass as bass
from concourse import mybir
from concourse.masks import make_identity

P = 128
F32 = mybir.dt.float32
BF16 = mybir.dt.bfloat16
AF = mybir.ActivationFunctionType
ALU = mybir.AluOpType
NEG = -1.0e30

B, T, D, H = 4, 1024, 1024, 16
dk = D // H          # 64
ET = D // P          # 8 e-tiles
DT = D // P          # 8 contraction tiles
TT = T // P          # 8 t-tiles
NQ = 4               # q-tiles per core
TQ = NQ * P          # 512 q rows per core
EXT = [2, 4, 6, 8]   # padded k-extents (tiles) per q-slot, both stripes
QT_A = [0, 3, 4, 7]
QT_B = [1, 2, 5, 6]
EPS = 1e-5


def emit(tc, io):
    nc = tc.nc
    from contextlib import ExitStack
    st = ExitStack()

    cpool = st.enter_context(tc.tile_pool(name="consts", bufs=1))
    ppool = st.enter_context(tc.tile_pool(name="persist", bufs=1))

    # ---------------- constants ----------------
    ident = cpool.tile([P, P], F32)
    make_identity(nc, ident)
    ident_bf = cpool.tile([P, P], BF16)
    nc.vector.tensor_copy(out=ident_bf, in_=ident)

    ones1_bf = cpool.tile([1, P], BF16)
    nc.vector.memset(ones1_bf, 1.0)

    # biases per-partition per-etile: bq_pe[p, et] = bq[et*128+p]
    bq_pe = cpool.tile([P, ET], F32)
    nc.sync.dma_start(out=bq_pe, in_=bass.AP(
        tensor=io["bq"].tensor, offset=io["bq"].offset, ap=[[1, P], [P, ET]]))
    bq8 = cpool.tile([P, ET], F32)
    nc.vector.tensor_scalar_mul(bq8, bq_pe, 0.125)

    stage = cpool.tile([1, 2 * D], F32)
    nc.sync.dma_start(out=stage[:, 0:D], in_=io["bv"][None, :])
    nc.sync.dma_start(out=stage[:, D:2 * D], in_=io["bo"][None, :])
    bv_bf = cpool.tile([1, D], BF16)
    nc.vector.tensor_copy(out=bv_bf, in_=stage[:, 0:D])
    bo_bf = cpool.tile([1, D], BF16)
    nc.vector.tensor_copy(out=bo_bf, in_=stage[:, D:2 * D])

    def bcast(src, cols, tag):
        t = cpool.tile([P, cols], F32, tag=tag)
        nc.sync.dma_start(out=t, in_=bass.AP(
            tensor=src.tensor, offset=src.offset, ap=[[0, P]] + src.ap))
        return t

    lng_bc = bcast(io["lng"], D, "lng_bc")
    lnb_bc = bcast(io["lnb"], D, "lnb_bc")
    gam_bc = bcast(io["gam"], H, "gam_bc")
    gneg = cpool.tile([P, H], F32)
    nc.scalar.activation(out=gneg, in_=gam_bc, func=AF.Abs)
    nc.vector.tensor_scalar_mul(gneg, gneg, -1.0)

    iota = cpool.tile([P, T], F32)        # k - qi
    nc.sync.dma_start(out=iota, in_=io["iota"])
    gcol = cpool.tile([P, NQ], F32)       # -gi*128 per slot
    nc.sync.dma_start(out=gcol, in_=io["gcol"])
    zfix = cpool.tile([P, NQ], F32)
    nc.sync.dma_start(out=zfix, in_=io["zfix"])
    maskme = cpool.tile([P, NQ * 2 * P], BF16)   # additive mask, last 2 tiles/slot
    mstage = cpool.tile([P, NQ * 2 * P], F32)
    nc.sync.dma_start(out=mstage, in_=io["maskme"])
    nc.vector.tensor_copy(out=maskme, in_=mstage)

    eps_col = cpool.tile([P, 1], F32)
    nc.vector.memset(eps_col, EPS)

    # ---------------- persistent activations ----------------
    kT = ppool.tile([P, ET, T], BF16, tag="kT")       # (e, t) per e-tile
    qT = ppool.tile([P, ET, TQ], BF16, tag="qT")      # (e, q) prescaled 1/8
    vb = ppool.tile([P, TT, D], BF16, tag="vb")       # (t, e) natural
    concT = ppool.tile([P, ET, TQ], BF16, tag="concT")

    # ---------------- projections ----------------
    with tc.tile_pool(name="wx", bufs=1) as wx, \
         tc.tile_pool(name="ppsum", bufs=1, space="PSUM") as pp, \
         tc.tile_pool(name="vpsum", bufs=2, space="PSUM") as vp:
        wqT_sb = wx.tile([P, DT, D], BF16, tag="wqT")
        wvT_sb = wx.tile([P, DT, D], BF16, tag="wvT")
        xkT_sb = wx.tile([P, DT, T], BF16, tag="xkT")
        xvT_sb = wx.tile([P, DT, T], BF16, tag="xvT")
        xqT_sb = wx.tile([P, DT, TQ], BF16, tag="xqT")
        for dc in range(DT):
            r = slice(dc * P, (dc + 1) * P)
            nc.sync.dma_start(out=wqT_sb[:, dc, :], in_=io["wqT"][r, :])
            nc.sync.dma_start(out=xkT_sb[:, dc, :], in_=io["xkT"][r, :])
            nc.sync.dma_start(out=xqT_sb[:, dc, :], in_=io["xqT"][r, :])
            nc.sync.dma_start(out=wvT_sb[:, dc, :], in_=io["wvT"][r, :])
            nc.sync.dma_start(out=xvT_sb[:, dc, :], in_=io["xvT"][r, :])

        # k/q projections, et-major; shared stationary weight per (et, dc)
        for et in range(ET):
            es = slice(et * P, (et + 1) * P)
            kq = pp.tile([P, T], F32, tag="kq")
            qq = pp.tile([P, TQ], F32, tag="qq")
            for dc in range(DT):
                lw = wqT_sb[:, dc, es]
                nc.tensor.matmul(kq[:, 0:512], lw, xkT_sb[:, dc, 0:512],
                                 start=(dc == 0), stop=(dc == DT - 1))
                nc.tensor.matmul(kq[:, 512:1024], lw, xkT_sb[:, dc, 512:1024],
                                 start=(dc == 0), stop=(dc == DT - 1))
                nc.tensor.matmul(qq, lw, xqT_sb[:, dc, :],
                                 start=(dc == 0), stop=(dc == DT - 1))
            nc.scalar.activation(out=kT[:, et, :], in_=kq, func=AF.Identity,
                                 bias=bq_pe[:, et:et + 1])
            nc.scalar.activation(out=qT[:, et, :], in_=qq, func=AF.Identity,
                                 bias=bq8[:, et:et + 1], scale=0.125)

        # v projection, tt-major
        for tt in range(TT):
            ts_ = slice(tt * P, (tt + 1) * P)
            for fh in range(2):
                fs = slice(fh * 512, (fh + 1) * 512)
                vv = vp.tile([P, 512], F32, tag="vv")
                for dc in range(DT):
                    nc.tensor.matmul(vv, xvT_sb[:, dc, ts_], wvT_sb[:, dc, fs],
                                     start=(dc == 0), stop=False)
                nc.tensor.matmul(vv, ones1_bf, bv_bf[:, fs],
                                 start=False, stop=True)
                nc.scalar.activation(out=vb[:, tt, fs], in_=vv, func=AF.Copy)

    # ---------------- late weights ----------------
    lpool = st.enter_context(tc.tile_pool(name="late", bufs=1))
    woT_sb = lpool.tile([P, DT, D], BF16, tag="woT")
    qnat = lpool.tile([P, NQ, D], F32, tag="qnat")
    for dc in range(DT):
        nc.sync.dma_start(out=woT_sb[:, dc, :],
                          in_=io["woT"][dc * P:(dc + 1) * P, :])
    for j in range(NQ):
        nc.sync.dma_start(out=qnat[:, j, :], in_=io["xq"][j * P:(j + 1) * P, :])

    # ---------------- attention + output ----------------
    with tc.tile_pool(name="anp", bufs=2) as anp, \
         tc.tile_pool(name="abig", bufs=3) as abig, \
         tc.tile_pool(name="asm", bufs=2) as asm, \
         tc.tile_pool(name="spsum", bufs=2, space="PSUM") as sp, \
         tc.tile_pool(name="tpsum", bufs=1, space="PSUM") as tp_pool, \
         tc.tile_pool(name="vpsum2", bufs=2, space="PSUM") as pvp, \
         tc.tile_pool(name="opsum", bufs=1, space="PSUM") as op_pool, \
         tc.tile_pool(name="otmp", bufs=2) as otmp, \
         tc.tile_pool(name="omini", bufs=2) as omini:

        for j in range(NQ):
            nkt = EXT[j]
            Lk = nkt * P
            qs = slice(j * P, (j + 1) * P)

            negp = anp.tile([P, T], F32, tag="negp")
            nc.vector.tensor_scalar(out=negp[:, :Lk], in0=iota[:, :Lk],
                                    scalar1=gcol[:, j:j + 1], scalar2=0.0,
                                    op0=ALU.add, op1=ALU.min)
            # negp broadcast over a head pair: [P, 2, Lk] with 0-stride
            negp2 = bass.AP(tensor=negp.tensor, offset=negp.offset,
                            ap=[negp.ap[0], [0, 2], [1, Lk]])
            lkcol = anp.tile([P, 1], F32, tag="lkcol")
            nc.vector.memset(lkcol, float(Lk))

            for hp in range(H // 2):
                h0 = 2 * hp
                et = hp  # = h0 // 2
                Zp = asm.tile([P, 2], F32, tag="Zp")
                gz = asm.tile([P, 2], F32, tag="gz")
                Z2p = asm.tile([P, 2], F32, tag="Z2p")
                m2p = asm.tile([P, 2], F32, tag="m2p")
                Ss, e2s = [], []
                x_ = abig.tile([P, 2, T], F32, tag="x")
                # --- stage A per head: scores, exp(e) for the suffix scan ---
                for hh in range(2):
                    po = hh * dk
                    S = sp.tile([P, T], F32, tag="S")
                    Ss.append(S)
                    c0 = 0
                    while c0 < Lk - 256:
                        w = min(512, Lk - 256 - c0)
                        nc.tensor.matmul(S[:, c0:c0 + w],
                                         qT[po:po + dk, et, qs],
                                         kT[po:po + dk, et, c0:c0 + w],
                                         start=True, stop=True)
                        c0 += w
                    nc.tensor.matmul(S[:, Lk - 256:Lk],
                                     qT[po:po + dk, et, qs],
                                     kT[po:po + dk, et, Lk - 256:Lk],
                                     start=True, stop=False)
                    nc.tensor.matmul(S[:, Lk - 256:Lk], ident_bf,
                                     maskme[:, j * 256:(j + 1) * 256],
                                     start=False, stop=True)
                    e_ = abig.tile([P, T], F32, tag="e")
                    nc.scalar.activation(out=e_[:, :Lk], in_=S[:, :Lk],
                                         func=AF.Exp,
                                         accum_out=Zp[:, hh:hh + 1])
                    nc.vector.memset(x_[:, hh, Lk - 1:Lk], 0.0)
                    nc.vector.tensor_tensor_scan(
                        out=x_[:, hh, Lk - 2::-1], data0=e_[:, Lk - 1:0:-1],
                        data1=e_[:, Lk - 1:0:-1], initial=0.0,
                        op0=ALU.add, op1=ALU.bypass)
                # --- pair: x2 = suffix * (i-k); sqrt via ln/exp (same table
                # set as Exp); gz = -|gamma| / sqrt(Z + zfix) ---
                x2 = bass.AP(tensor=x_.tensor, offset=x_.offset,
                             ap=[x_.ap[0], [T, 2], [1, Lk]])
                nc.vector.scalar_tensor_tensor(
                    out=x2, in0=x2, scalar=-1.0,
                    in1=negp2, op0=ALU.mult, op1=ALU.mult)
                nc.scalar.activation(out=x2, in_=x2, func=AF.Ln)
                nc.scalar.activation(out=x2, in_=x2, func=AF.Exp, scale=0.5)
                nc.vector.tensor_tensor(out=Zp, in0=Zp,
                                        in1=zfix[:, j:j + 1].to_broadcast([P, 2]),
                                        op=ALU.add)
                nc.scalar.activation(out=gz, in_=Zp, func=AF.Ln)
                nc.scalar.activation(out=gz, in_=gz, func=AF.Exp, scale=-0.5)
                nc.vector.tensor_tensor(out=gz, in0=gz,
                                        in1=gneg[:, h0:h0 + 2], op=ALU.mult)
                # --- stage B per head: f, S2 = S*f (psum), e2, rowmax ---
                for hh in range(2):
                    S = Ss[hh]
                    f_ = abig.tile([P, T], F32, tag="f")
                    nc.scalar.activation(out=f_[:, :Lk], in_=x_[:, hh, :Lk],
                                         func=AF.Exp, scale=gz[:, hh:hh + 1])
                    nc.vector.scalar_tensor_tensor(
                        out=S[:, :Lk], in0=f_[:, :Lk], scalar=1.0,
                        in1=S[:, :Lk], op0=ALU.mult, op1=ALU.mult)
                    e2 = abig.tile([P, T], BF16, tag="e2")
                    e2s.append(e2)
                    nc.scalar.activation(out=e2[:, :Lk], in_=S[:, :Lk],
                                         func=AF.Exp,
                                         accum_out=Z2p[:, hh:hh + 1])
                    nc.vector.tensor_reduce(
                        out=m2p[:, hh:hh + 1], in_=e2[:, :Lk],
                        axis=mybir.AxisListType.X, op=ALU.max)
                # --- pair tiny: cc = min(Z2'/m2, 5) / Z2' ---
                nc.vector.tensor_tensor(out=Z2p, in0=Z2p,
                                        in1=zfix[:, j:j + 1].to_broadcast([P, 2]),
                                        op=ALU.add)
                nc.vector.tensor_tensor(out=m2p, in0=m2p,
                                        in1=zfix[:, j:j + 1].to_broadcast([P, 2]),
                                        op=ALU.add)
                cc = asm.tile([P, 2], F32, tag="cc")
                rz2 = asm.tile([P, 2], F32, tag="rz2")
                nc.vector.reciprocal(out=rz2, in_=Z2p)
                rm2 = asm.tile([P, 2], F32, tag="rm2")
                nc.vector.reciprocal(out=rm2, in_=m2p)
                nc.vector.tensor_tensor(out=cc, in0=Z2p, in1=rm2, op=ALU.mult)
                nc.vector.tensor_scalar_min(cc, cc, 5.0)
                nc.vector.tensor_tensor(out=cc, in0=cc, in1=rz2, op=ALU.mult)
                # --- stage C per head: rescale, transpose, PV ---
                pv = pvp.tile([dk, 2 * P], F32, tag="pv")
                for hh in range(2):
                    h = h0 + hh
                    e2 = e2s[hh]
                    p_ = asm.tile([P, T], BF16, tag="p")
                    nc.vector.tensor_scalar_mul(p_[:, :Lk], e2[:, :Lk],
                                                cc[:, hh:hh + 1])
                    pT = asm.tile([P, T], BF16, tag="pT")
                    for g0 in range(0, nkt, 4):
                        gn = min(4, nkt - g0)
                        tp = tp_pool.tile([P, 4 * P], BF16, tag="tp")
                        for gg in range(gn):
                            kt = g0 + gg
                            nc.tensor.transpose(tp[:, gg * P:(gg + 1) * P],
                                                p_[:, kt * P:(kt + 1) * P],
                                                ident_bf)
                        nc.vector.tensor_copy(out=pT[:, g0 * P:(g0 + gn) * P],
                                              in_=tp[:, :gn * P])
                    for kt in range(nkt):
                        nc.tensor.matmul(pv[:, hh * P:(hh + 1) * P],
                                         vb[:, kt, h * dk:(h + 1) * dk],
                                         pT[:, kt * P:(kt + 1) * P],
                                         start=(kt == 0), stop=(kt == nkt - 1))
                for hh in range(2):
                    nc.scalar.activation(
                        out=concT[hh * dk:(hh + 1) * dk, et, qs],
                        in_=pv[:, hh * P:(hh + 1) * P], func=AF.Copy)

            # ---- output projection + residual + layernorm for q-tile j ----
            xsb = otmp.tile([P, D], F32, tag="xsb")
            for fh in range(2):
                fs = slice(fh * 512, (fh + 1) * 512)
                ps = op_pool.tile([P, 512], F32, tag="op")
                for et2 in range(ET):
                    nc.tensor.matmul(ps, concT[:, et2, qs],
                                     woT_sb[:, et2, fs],
                                     start=(et2 == 0), stop=False)
                nc.tensor.matmul(ps, ones1_bf, bo_bf[:, fs],
                                 start=False, stop=True)
                nc.vector.tensor_tensor(out=xsb[:, fs], in0=ps,
                                        in1=qnat[:, j, fs], op=ALU.add)
            stats = omini.tile([P, 2, 6], F32, tag="stats")
            for sg in range(2):
                nc.vector.bn_stats(out=stats[:, sg, :],
                                   in_=xsb[:, sg * 512:(sg + 1) * 512])
            mv = omini.tile([P, 2], F32, tag="mv")
            nc.vector.bn_aggr(out=mv, in_=stats)
            rstd = omini.tile([P, 1], F32, tag="rstd")
            nc.scalar.activation(out=rstd, in_=mv[:, 1:2], func=AF.Ln,
                                 bias=eps_col)
            nc.scalar.activation(out=rstd, in_=rstd, func=AF.Exp, scale=-0.5)
            nmr = omini.tile([P, 1], F32, tag="nmr")
            nc.vector.scalar_tensor_tensor(out=nmr, in0=mv[:, 0:1], scalar=-1.0,
                                           in1=rstd, op0=ALU.mult, op1=ALU.mult)
            ysb = otmp.tile([P, D], F32, tag="ysb")
            nc.scalar.activation(out=ysb, in_=xsb, func=AF.Identity,
                                 bias=nmr, scale=rstd)
            nc.vector.tensor_tensor(out=ysb, in0=ysb, in1=lng_bc, op=ALU.mult)
            nc.vector.tensor_tensor(out=ysb, in0=ysb, in1=lnb_bc, op=ALU.add)
            nc.sync.dma_start(out=io["y"][qs, :], in_=ysb)

    st.close()


# ------------------------------------------------------------------
# program build + host-side runner
# ------------------------------------------------------------------

def build_program():
    nc = bacc.Bacc("TRN2", target_bir_lowering=False, debug=False,
                   num_devices=8)
    io = {}

    def inp(name, shape, dt=F32):
        io[name] = nc.dram_tensor(name, shape, dt, kind="ExternalInput").ap()

    inp("wqT", [D, D], BF16)
    inp("wvT", [D, D], BF16)
    inp("woT", [D, D], BF16)
    inp("xkT", [D, T], BF16)
    inp("xvT", [D, T], BF16)
    inp("xqT", [D, TQ], BF16)
    inp("xq", [TQ, D])
    inp("bq", [D])
    inp("bv", [D])
    inp("bo", [D])
    inp("gam", [H])
    inp("lng", [D])
    inp("lnb", [D])
    inp("iota", [P, T])
    inp("gcol", [P, NQ])
    inp("zfix", [P, NQ])
    inp("maskme", [P, NQ * 2 * P])
    io["y"] = nc.dram_tensor("y", [TQ, D], F32, kind="ExternalOutput").ap()
    with tile.TileContext(nc) as tc:
        emit(tc, io)
    nc.compile()
    _unify_act_tables(nc)
    return nc


def _unify_act_tables(nc):
    """Retarget every ACT table load to natural_log_exp_and_others (which
    contains all functions this kernel uses: exp/ln/identity/copy/abs) and
    drop now-redundant consecutive loads.  The default chooser alternates
    exp_and_others <-> natural_log, costing ~2.7us per switch."""
    from concourse.hw_specs import get_activation_tables
    tables = get_activation_tables(nc.m.arch)
    names = list(tables.keys())
    target = names.index("natural_log_exp_and_others")
    allowed = tables["natural_log_exp_and_others"]
    used = set()
    for fn in nc.m.functions:
        for b in fn.blocks:
            for ins in b.instructions:
                if isinstance(ins, mybir.InstActivation):
                    used.add(ins.func)
    if not used <= allowed:
        return  # some function outside the combined set; keep default loads
    for fn in nc.m.functions:
        for b in fn.blocks:
            new = []
            cur = -1
            for ins in b.instructions:
                if (isinstance(ins, mybir.InstLoadActFuncSet)
                        and ins.sync_info is None):
                    ins.act_func_set_id = target
                    if cur == target:
                        continue
                    cur = target
                new.append(ins)
            b.instructions[:] = new


def make_in_maps(inputs):
    import ml_dtypes
    bf = ml_dtypes.bfloat16
    q = np.asarray(inputs["query"], np.float32)
    k = np.asarray(inputs["key"], np.float32)
    v = np.asarray(inputs["values"], np.float32)
    wqT = np.ascontiguousarray(np.asarray(inputs["Wq"], np.float32).T).astype(bf)
    wvT = np.ascontiguousarray(np.asarray(inputs["Wv"], np.float32).T).astype(bf)
    woT = np.ascontiguousarray(np.asarray(inputs["Wo"], np.float32).T).astype(bf)
    small = {
        "bq": np.ascontiguousarray(inputs["bq"], np.float32),
        "bv": np.ascontiguousarray(inputs["bv"], np.float32),
        "bo": np.ascontiguousarray(inputs["bo"], np.float32),
        "gam": np.ascontiguousarray(inputs["gammas"], np.float32),
        "lng": np.ascontiguousarray(inputs["ln_g"], np.float32),
        "lnb": np.ascontiguousarray(inputs["ln_b"], np.float32),
    }
    iota = (np.arange(T)[None, :] - np.arange(P)[:, None]).astype(np.float32)

    stripe_data = []
    for qtiles in (QT_A, QT_B):
        rows = np.concatenate([np.arange(g * P, (g + 1) * P) for g in qtiles])
        gcol = np.zeros((P, NQ), np.float32)
        zfix = np.zeros((P, NQ), np.float32)
        maskme = np.zeros((P, NQ, 2, P), np.float32)
        for jj, gi in enumerate(qtiles):
            gcol[:, jj] = -float(gi * P)
            if gi == 0:
                zfix[0, jj] = 1.0
            i_glob = gi * P + np.arange(P)[:, None]
            for tt in range(2):
                tpos = EXT[jj] - 2 + tt
                kk = tpos * P + np.arange(P)[None, :]
                maskme[:, jj, tt, :] = np.where(kk >= i_glob, NEG, 0.0)
        stripe_data.append(dict(
            rows=rows, gcol=gcol, zfix=zfix,
            maskme=maskme.reshape(P, NQ * 2 * P)))

    maps = []
    for c in range(8):
        sd = stripe_data[c // 4]
        b = c % 4
        rows = sd["rows"]
        m = dict(small)
        m["wqT"], m["wvT"], m["woT"] = wqT, wvT, woT
        m["xkT"] = np.ascontiguousarray(k[b].T).astype(bf)
        m["xvT"] = np.ascontiguousarray(v[b].T).astype(bf)
        m["xqT"] = np.ascontiguousarray(q[b].T[:, rows]).astype(bf)
        m["xq"] = np.ascontiguousarray(q[b][rows])
        m["iota"] = iota
        m["gcol"] = sd["gcol"]
        m["zfix"] = sd["zfix"]
        m["maskme"] = sd["maskme"]
        maps.append(m)
    return maps


class _Runner:
    def __init__(self):
        self.nc = build_program()
        self._fn = None

    def _make_fn(self, nc, devices):
        import jax
        from jax.sharding import Mesh, PartitionSpec
        from jax.experimental.shard_map import shard_map
        from concourse import bass2jax
        from concourse.bass2jax import _bass_exec_p, partition_id_tensor

        bass2jax.install_neuronx_cc_hook()
        partition_name = (nc.partition_id_tensor.name
                          if nc.partition_id_tensor else None)
        in_names, out_names, out_avals, zero_outs = [], [], [], []
        for alloc in nc.m.functions[0].allocations:
            if not isinstance(alloc, mybir.MemoryLocationSet):
                continue
            name = alloc.memorylocations[0].name
            if alloc.kind == "ExternalInput":
                if name != partition_name:
                    in_names.append(name)
            elif alloc.kind == "ExternalOutput":
                shape = tuple(alloc.tensor_shape)
                dtype = mybir.dt.np(alloc.dtype)
                out_names.append(name)
                out_avals.append(jax.core.ShapedArray(shape, dtype))
                zero_outs.append(np.zeros(shape, dtype))
        n_params = len(in_names)
        all_in = list(in_names) + list(out_names)
        if partition_name is not None:
            all_in.append(partition_name)

        def _body(*args):
            operands = list(args)
            if partition_name is not None:
                operands.append(partition_id_tensor())
            outs = _bass_exec_p.bind(
                *operands, out_avals=tuple(out_avals), in_names=tuple(all_in),
                out_names=tuple(out_names), lowering_input_output_aliases=(),
                sim_require_finite=True, sim_require_nnan=True, nc=nc)
            return tuple(outs)

        mesh = Mesh(np.asarray(devices), ("core",))
        n = n_params + len(out_names)
        fn = jax.jit(shard_map(_body, mesh=mesh,
                               in_specs=(PartitionSpec("core"),) * n,
                               out_specs=(PartitionSpec("core"),) * len(out_names),
                               check_rep=False),
                     keep_unused=True)
        return fn, in_names, out_names, zero_outs

    def fn(self):
        if self._fn is None:
            import jax
            self._fn = self._make_fn(self.nc, jax.devices()[:8])
        return self._fn

    def run(self, inputs):
        import jax
        fn, in_names, out_names, zero_outs = self.fn()
        maps = make_in_maps(inputs)
        args = [np.concatenate([np.asarray(m[nm]) for m in maps], axis=0)
                for nm in in_names]
        args += [np.zeros((8 * z.shape[0], *z.shape[1:]), z.dtype)
                 for z in zero_outs]
        outs = fn(*args)
        jax.block_until_ready(outs)
        y = np.asarray(outs[0]).reshape(8, TQ, D)
        out = np.empty((B, T, D), np.float32)
        for c in range(8):
            qtiles = (QT_A, QT_B)[c // 4]
            b = c % 4
            for jj, g in enumerate(qtiles):
                out[b, g * P:(g + 1) * P] = y[c, jj * P:(jj + 1) * P]
        return out


_runner = None


def kernel(**inputs) -> np.ndarray:
    global _runner
    if _runner is None:
        _runner = _Runner()
    return _runner.run(inputs)


# revision 20
# speedup vs baseline: 7433.8527x; 1.0399x over previous
"""Trainium2 Bass kernel for nn_DTransformer (sparse decay attention layer).

Single SPMD program on 8 cores: core c -> (stripe = c//4, batch = c%4).
Stripe A owns q-tiles {0,3,4,7}, stripe B {1,2,5,6} (of 8 tiles of 128
rows).  Both stripes process their j-th tile with a PADDED causal extent
EXT[j] = (2,4,6,8) k-tiles so the instruction stream is identical; the
true causal structure is injected via per-core input data (additive mask
tiles, iota offsets, zfix columns).

Math per (q-tile, head), S = (q@k^T)/sqrt(dk) with strict-causal mask:
  e    = exp(S)                       [ACT, accum -> Z]
  suf  = reversed-exclusive-cumsum(e) [DVE scan, neg-stride APs]
  sq   = suf^0.5 * sqrt(i-k)          [DVE stt pow+mult; sqrt((Z-cum)*pos)
                                       factorized]
  f    = exp(gz * sq), gz = -|gamma|/sqrt(Z)   [ACT, per-row scale]
  e2   = e^f = exp(S*f)               [DVE stt pow, accum -> Z2, bf16]
  m2   = rowmax(e2)                   [DVE tensor_mask_reduce]
  p    = e2 * min(Z2/m2,5)/Z2         [DVE tensor_scalar, bf16]
  out += p @ v                        [PE: transpose p + matmul]
Projections / output proj / residual / LayerNorm standard; weights and
activations are fed pre-transposed and bf16-cast from the host.
"""

import numpy as np

import concourse.bacc as bacc
import concourse.tile as tile
import concourse.bass as bass
from concourse import mybir
from concourse.masks import make_identity

P = 128
F32 = mybir.dt.float32
BF16 = mybir.dt.bfloat16
AF = mybir.ActivationFunctionType
ALU = mybir.AluOpType
NEG = -1.0e30

B, T, D, H = 4, 1024, 1024, 16
dk = D // H          # 64
ET = D // P          # 8 e-tiles
DT = D // P          # 8 contraction tiles
TT = T // P          # 8 t-tiles
NQ = 4               # q-tiles per core
TQ = NQ * P          # 512 q rows per core
EXT = [2, 4, 6, 8]   # padded k-extents (tiles) per q-slot, both stripes
QT_A = [0, 3, 4, 7]
QT_B = [1, 2, 5, 6]
EPS = 1e-5


def emit(tc, io):
    nc = tc.nc
    from contextlib import ExitStack
    st = ExitStack()

    cpool = st.enter_context(tc.tile_pool(name="consts", bufs=1))
    ppool = st.enter_context(tc.tile_pool(name="persist", bufs=1))

    # ---------------- constants ----------------
    ident = cpool.tile([P, P], F32)
    make_identity(nc, ident)
    ident_bf = cpool.tile([P, P], BF16)
    nc.vector.tensor_copy(out=ident_bf, in_=ident)

    ones1_bf = cpool.tile([1, P], BF16)
    nc.vector.memset(ones1_bf, 1.0)

    # biases per-partition per-etile: bq_pe[p, et] = bq[et*128+p]
    bq_pe = cpool.tile([P, ET], F32)
    nc.sync.dma_start(out=bq_pe, in_=bass.AP(
        tensor=io["bq"].tensor, offset=io["bq"].offset, ap=[[1, P], [P, ET]]))
    bq8 = cpool.tile([P, ET], F32)
    nc.vector.tensor_scalar_mul(bq8, bq_pe, 0.125)

    bv_bf = cpool.tile([1, D], BF16)
    nc.sync.dma_start(out=bv_bf, in_=io["bv"][None, :])
    bo_bf = cpool.tile([1, D], BF16)
    nc.sync.dma_start(out=bo_bf, in_=io["bo"][None, :])

    def bcast(src, cols, tag):
        t = cpool.tile([P, cols], F32, tag=tag)
        nc.sync.dma_start(out=t, in_=bass.AP(
            tensor=src.tensor, offset=src.offset, ap=[[0, P]] + src.ap))
        return t

    lng_bc = bcast(io["lng"], D, "lng_bc")
    lnb_bc = bcast(io["lnb"], D, "lnb_bc")
    gam_bc = bcast(io["gam"], H, "gam_bc")
    gneg = cpool.tile([P, H], F32)
    nc.scalar.activation(out=gneg, in_=gam_bc, func=AF.Abs)
    nc.vector.tensor_scalar_mul(gneg, gneg, -1.0)

    iota = cpool.tile([P, T], F32)        # k - qi
    nc.sync.dma_start(out=iota, in_=io["iota"])
    gcol = cpool.tile([P, NQ], F32)       # -gi*128 per slot
    nc.sync.dma_start(out=gcol, in_=io["gcol"])
    zfix = cpool.tile([P, NQ], F32)
    nc.sync.dma_start(out=zfix, in_=io["zfix"])
    maskme = cpool.tile([P, NQ * 2 * P], BF16)   # additive mask, last 2 tiles/slot
    nc.sync.dma_start(out=maskme, in_=io["maskme"])

    eps_col = cpool.tile([P, 1], F32)
    nc.vector.memset(eps_col, EPS)

    # ---------------- persistent activations ----------------
    kT = ppool.tile([P, ET, T], BF16, tag="kT")       # (e, t) per e-tile
    qT = ppool.tile([P, ET, TQ], BF16, tag="qT")      # (e, q) prescaled 1/8
    vb = ppool.tile([P, TT, D], BF16, tag="vb")       # (t, e) natural
    concT = ppool.tile([P, ET, TQ], BF16, tag="concT")

    # ---------------- projections ----------------
    with tc.tile_pool(name="wx", bufs=1) as wx, \
         tc.tile_pool(name="ppsum", bufs=1, space="PSUM") as pp, \
         tc.tile_pool(name="vpsum", bufs=2, space="PSUM") as vp:
        wqT_sb = wx.tile([P, DT, D], BF16, tag="wqT")
        wvT_sb = wx.tile([P, DT, D], BF16, tag="wvT")
        xkT_sb = wx.tile([P, DT, T], BF16, tag="xkT")
        xvT_sb = wx.tile([P, DT, T], BF16, tag="xvT")
        xqT_sb = wx.tile([P, DT, TQ], BF16, tag="xqT")
        for dc in range(DT):
            r = slice(dc * P, (dc + 1) * P)
            nc.sync.dma_start(out=wqT_sb[:, dc, :], in_=io["wqT"][r, :])
            nc.sync.dma_start(out=xkT_sb[:, dc, :], in_=io["xkT"][r, :])
            nc.sync.dma_start(out=xqT_sb[:, dc, :], in_=io["xqT"][r, :])
            nc.sync.dma_start(out=wvT_sb[:, dc, :], in_=io["wvT"][r, :])
            nc.sync.dma_start(out=xvT_sb[:, dc, :], in_=io["xvT"][r, :])

        # k/q projections, et-major; shared stationary weight per (et, dc)
        for et in range(ET):
            es = slice(et * P, (et + 1) * P)
            kq = pp.tile([P, T], F32, tag="kq")
            qq = pp.tile([P, TQ], F32, tag="qq")
            for dc in range(DT):
                lw = wqT_sb[:, dc, es]
                nc.tensor.matmul(kq[:, 0:512], lw, xkT_sb[:, dc, 0:512],
                                 start=(dc == 0), stop=(dc == DT - 1))
                nc.tensor.matmul(kq[:, 512:1024], lw, xkT_sb[:, dc, 512:1024],
                                 start=(dc == 0), stop=(dc == DT - 1))
                nc.tensor.matmul(qq, lw, xqT_sb[:, dc, :],
                                 start=(dc == 0), stop=(dc == DT - 1))
            nc.scalar.activation(out=kT[:, et, :], in_=kq, func=AF.Identity,
                                 bias=bq_pe[:, et:et + 1])
            nc.scalar.activation(out=qT[:, et, :], in_=qq, func=AF.Identity,
                                 bias=bq8[:, et:et + 1], scale=0.125)

        # v projection, tt-major
        for tt in range(TT):
            ts_ = slice(tt * P, (tt + 1) * P)
            for fh in range(2):
                fs = slice(fh * 512, (fh + 1) * 512)
                vv = vp.tile([P, 512], F32, tag="vv")
                for dc in range(DT):
                    nc.tensor.matmul(vv, xvT_sb[:, dc, ts_], wvT_sb[:, dc, fs],
                                     start=(dc == 0), stop=False)
                nc.tensor.matmul(vv, ones1_bf, bv_bf[:, fs],
                                 start=False, stop=True)
                nc.scalar.activation(out=vb[:, tt, fs], in_=vv, func=AF.Copy)

    # ---------------- late weights ----------------
    lpool = st.enter_context(tc.tile_pool(name="late", bufs=1))
    woT_sb = lpool.tile([P, DT, D], BF16, tag="woT")
    qnat = lpool.tile([P, NQ, D], F32, tag="qnat")
    for dc in range(DT):
        nc.sync.dma_start(out=woT_sb[:, dc, :],
                          in_=io["woT"][dc * P:(dc + 1) * P, :])
    for j in range(NQ):
        nc.sync.dma_start(out=qnat[:, j, :], in_=io["xq"][j * P:(j + 1) * P, :])

    # ---------------- attention + output ----------------
    with tc.tile_pool(name="anp", bufs=2) as anp, \
         tc.tile_pool(name="abig", bufs=1) as abig, \
         tc.tile_pool(name="asm", bufs=2) as asm, \
         tc.tile_pool(name="spsum", bufs=3, space="PSUM") as sp, \
         tc.tile_pool(name="tpsum", bufs=1, space="PSUM") as tp_pool, \
         tc.tile_pool(name="otmp", bufs=2) as otmp, \
         tc.tile_pool(name="omini", bufs=2) as omini:

        for j in range(NQ):
            nkt = EXT[j]
            Lk = nkt * P
            qs = slice(j * P, (j + 1) * P)

            negp = anp.tile([P, T], F32, tag="negp")
            nc.vector.tensor_scalar(out=negp[:, :Lk], in0=iota[:, :Lk],
                                    scalar1=gcol[:, j:j + 1], scalar2=0.0,
                                    op0=ALU.add, op1=ALU.min)
            # negp broadcast over a head pair: [P, 2, Lk] with 0-stride
            negp2 = bass.AP(tensor=negp.tensor, offset=negp.offset,
                            ap=[negp.ap[0], [0, 2], [1, Lk]])

            for hp in range(H // 2):
                h0 = 2 * hp
                et = hp  # = h0 // 2
                Zp = asm.tile([P, 2], F32, tag="Zp")
                gz = asm.tile([P, 2], F32, tag="gz")
                Z2p = asm.tile([P, 2], F32, tag="Z2p")
                m2p = asm.tile([P, 2], F32, tag="m2p")
                Ss, e2s = [], []
                x_ = abig.tile([P, 2, T], F32, tag="x", bufs=3)
                # --- stage A per head: scores, exp(e) for the suffix scan ---
                for hh in range(2):
                    po = hh * dk
                    S = sp.tile([P, T], F32, tag="S")
                    Ss.append(S)
                    c0 = 0
                    while c0 < Lk - 256:
                        w = min(512, Lk - 256 - c0)
                        nc.tensor.matmul(S[:, c0:c0 + w],
                                         qT[po:po + dk, et, qs],
                                         kT[po:po + dk, et, c0:c0 + w],
                                         start=True, stop=True)
                        c0 += w
                    nc.tensor.matmul(S[:, Lk - 256:Lk],
                                     qT[po:po + dk, et, qs],
                                     kT[po:po + dk, et, Lk - 256:Lk],
                                     start=True, stop=False)
                    nc.tensor.matmul(S[:, Lk - 256:Lk], ident_bf,
                                     maskme[:, j * 256:(j + 1) * 256],
                                     start=False, stop=True)
                    e_ = abig.tile([P, T], F32, tag="e", bufs=5)
                    nc.scalar.activation(out=e_[:, :Lk], in_=S[:, :Lk],
                                         func=AF.Exp,
                                         accum_out=Zp[:, hh:hh + 1])
                    nc.vector.memset(x_[:, hh, Lk - 1:Lk], 0.0)
                    nc.vector.tensor_tensor_scan(
                        out=x_[:, hh, Lk - 2::-1], data0=e_[:, Lk - 1:0:-1],
                        data1=e_[:, Lk - 1:0:-1], initial=0.0,
                        op0=ALU.add, op1=ALU.bypass)
                # --- pair: x2 = suffix * (i-k); sqrt via ln/exp (same table
                # set as Exp); gz = -|gamma| / sqrt(Z + zfix) ---
                x2 = bass.AP(tensor=x_.tensor, offset=x_.offset,
                             ap=[x_.ap[0], [T, 2], [1, Lk]])
                nc.vector.scalar_tensor_tensor(
                    out=x2, in0=x2, scalar=-1.0,
                    in1=negp2, op0=ALU.mult, op1=ALU.mult)
                nc.scalar.activation(out=x2, in_=x2, func=AF.Ln)
                nc.scalar.activation(out=x2, in_=x2, func=AF.Exp, scale=0.5)
                nc.vector.tensor_tensor(out=Zp, in0=Zp,
                                        in1=zfix[:, j:j + 1].to_broadcast([P, 2]),
                                        op=ALU.add)
                nc.scalar.activation(out=gz, in_=Zp, func=AF.Ln)
                nc.scalar.activation(out=gz, in_=gz, func=AF.Exp, scale=-0.5)
                nc.vector.tensor_tensor(out=gz, in0=gz,
                                        in1=gneg[:, h0:h0 + 2], op=ALU.mult)
                # --- stage B per head: f, S2 = S*f (psum), e2, rowmax ---
                for hh in range(2):
                    S = Ss[hh]
                    f_ = abig.tile([P, T], F32, tag="f", bufs=3)
                    nc.scalar.activation(out=f_[:, :Lk], in_=x_[:, hh, :Lk],
                                         func=AF.Exp, scale=gz[:, hh:hh + 1])
                    nc.vector.scalar_tensor_tensor(
                        out=S[:, :Lk], in0=f_[:, :Lk], scalar=1.0,
                        in1=S[:, :Lk], op0=ALU.mult, op1=ALU.mult)
                    e2 = abig.tile([P, T], BF16, tag="e2", bufs=4)
                    e2s.append(e2)
                    nc.scalar.activation(out=e2[:, :Lk], in_=S[:, :Lk],
                                         func=AF.Exp,
                                         accum_out=Z2p[:, hh:hh + 1])
                    nc.vector.tensor_reduce(
                        out=m2p[:, hh:hh + 1], in_=e2[:, :Lk],
                        axis=mybir.AxisListType.X, op=ALU.max)
                # --- pair tiny: cc = min(Z2'/m2, 5) / Z2' ---
                nc.vector.tensor_tensor(out=Z2p, in0=Z2p,
                                        in1=zfix[:, j:j + 1].to_broadcast([P, 2]),
                                        op=ALU.add)
                nc.vector.tensor_tensor(out=m2p, in0=m2p,
                                        in1=zfix[:, j:j + 1].to_broadcast([P, 2]),
                                        op=ALU.add)
                cc = asm.tile([P, 2], F32, tag="cc")
                rz2 = asm.tile([P, 2], F32, tag="rz2")
                nc.vector.reciprocal(out=rz2, in_=Z2p)
                rm2 = asm.tile([P, 2], F32, tag="rm2")
                nc.vector.reciprocal(out=rm2, in_=m2p)
                nc.vector.tensor_tensor(out=cc, in0=Z2p, in1=rm2, op=ALU.mult)
                nc.vector.tensor_scalar_min(cc, cc, 5.0)
                nc.vector.tensor_tensor(out=cc, in0=cc, in1=rz2, op=ALU.mult)
                # --- stage C per head: rescale, transpose, PV ---
                opv = tp_pool.tile([P, 4 * P], F32, tag="opv")
                pv = opv[0:dk, 0:2 * P]
                for hh in range(2):
                    h = h0 + hh
                    e2 = e2s[hh]
                    p_ = asm.tile([P, T], BF16, tag="p")
                    nc.vector.tensor_scalar_mul(p_[:, :Lk], e2[:, :Lk],
                                                cc[:, hh:hh + 1])
                    pT = asm.tile([P, T], BF16, tag="pT")
                    for g0 in range(0, nkt, 4):
                        gn = min(4, nkt - g0)
                        tp = tp_pool.tile([P, 4 * P], BF16, tag="tp")
                        for gg in range(gn):
                            kt = g0 + gg
                            nc.tensor.transpose(tp[:, gg * P:(gg + 1) * P],
                                                p_[:, kt * P:(kt + 1) * P],
                                                ident_bf)
                        nc.vector.tensor_copy(out=pT[:, g0 * P:(g0 + gn) * P],
                                              in_=tp[:, :gn * P])
                    for kt in range(nkt):
                        nc.tensor.matmul(pv[:, hh * P:(hh + 1) * P],
                                         vb[:, kt, h * dk:(h + 1) * dk],
                                         pT[:, kt * P:(kt + 1) * P],
                                         start=(kt == 0), stop=(kt == nkt - 1))
                for hh in range(2):
                    nc.scalar.activation(
                        out=concT[hh * dk:(hh + 1) * dk, et, qs],
                        in_=pv[:, hh * P:(hh + 1) * P], func=AF.Copy)

            # ---- output projection + residual + layernorm for q-tile j ----
            xsb = otmp.tile([P, D], F32, tag="xsb")
            for fh in range(2):
                fs = slice(fh * 512, (fh + 1) * 512)
                ps = tp_pool.tile([P, 4 * P], F32, tag="opv")
                for et2 in range(ET):
                    nc.tensor.matmul(ps, concT[:, et2, qs],
                                     woT_sb[:, et2, fs],
                                     start=(et2 == 0), stop=False)
                nc.tensor.matmul(ps, ones1_bf, bo_bf[:, fs],
                                 start=False, stop=True)
                nc.vector.tensor_tensor(out=xsb[:, fs], in0=ps,
                                        in1=qnat[:, j, fs], op=ALU.add)
            stats = omini.tile([P, 2, 6], F32, tag="stats")
            for sg in range(2):
                nc.vector.bn_stats(out=stats[:, sg, :],
                                   in_=xsb[:, sg * 512:(sg + 1) * 512])
            mv = omini.tile([P, 2], F32, tag="mv")
            nc.vector.bn_aggr(out=mv, in_=stats)
            rstd = omini.tile([P, 1], F32, tag="rstd")
            nc.scalar.activation(out=rstd, in_=mv[:, 1:2], func=AF.Ln,
                                 bias=eps_col)
            nc.scalar.activation(out=rstd, in_=rstd, func=AF.Exp, scale=-0.5)
            nmr = omini.tile([P, 1], F32, tag="nmr")
            nc.vector.scalar_tensor_tensor(out=nmr, in0=mv[:, 0:1], scalar=-1.0,
                                           in1=rstd, op0=ALU.mult, op1=ALU.mult)
            ysb = otmp.tile([P, D], F32, tag="ysb")
            nc.scalar.activation(out=ysb, in_=xsb, func=AF.Identity,
                                 bias=nmr, scale=rstd)
            nc.vector.tensor_tensor(out=ysb, in0=ysb, in1=lng_bc, op=ALU.mult)
            nc.vector.tensor_tensor(out=ysb, in0=ysb, in1=lnb_bc, op=ALU.add)
            nc.sync.dma_start(out=io["y"][qs, :], in_=ysb)

    st.close()


# ------------------------------------------------------------------
# program build + host-side runner
# ------------------------------------------------------------------

def build_program():
    nc = bacc.Bacc("TRN2", target_bir_lowering=False, debug=False,
                   num_devices=8)
    io = {}

    def inp(name, shape, dt=F32):
        io[name] = nc.dram_tensor(name, shape, dt, kind="ExternalInput").ap()

    inp("wqT", [D, D], BF16)
    inp("wvT", [D, D], BF16)
    inp("woT", [D, D], BF16)
    inp("xkT", [D, T], BF16)
    inp("xvT", [D, T], BF16)
    inp("xqT", [D, TQ], BF16)
    inp("xq", [TQ, D])
    inp("bq", [D])
    inp("bv", [D], BF16)
    inp("bo", [D], BF16)
    inp("gam", [H])
    inp("lng", [D])
    inp("lnb", [D])
    inp("iota", [P, T])
    inp("gcol", [P, NQ])
    inp("zfix", [P, NQ])
    inp("maskme", [P, NQ * 2 * P], BF16)
    io["y"] = nc.dram_tensor("y", [TQ, D], F32, kind="ExternalOutput").ap()
    with tile.TileContext(nc) as tc:
        emit(tc, io)
    nc.compile()
    _unify_act_tables(nc)
    return nc


def _unify_act_tables(nc):
    """Retarget every ACT table load to natural_log_exp_and_others (which
    contains all functions this kernel uses: exp/ln/identity/copy/abs) and
    drop now-redundant consecutive loads.  The default chooser alternates
    exp_and_others <-> natural_log, costing ~2.7us per switch."""
    from concourse.hw_specs import get_activation_tables
    tables = get_activation_tables(nc.m.arch)
    names = list(tables.keys())
    target = names.index("natural_log_exp_and_others")
    allowed = tables["natural_log_exp_and_others"]
    used = set()
    for fn in nc.m.functions:
        for b in fn.blocks:
            for ins in b.instructions:
                if isinstance(ins, mybir.InstActivation):
                    used.add(ins.func)
    if not used <= allowed:
        return  # some function outside the combined set; keep default loads
    for fn in nc.m.functions:
        for b in fn.blocks:
            new = []
            cur = -1
            for ins in b.instructions:
                if (isinstance(ins, mybir.InstLoadActFuncSet)
                        and ins.sync_info is None):
                    ins.act_func_set_id = target
                    if cur == target:
                        continue
                    cur = target
                new.append(ins)
            b.instructions[:] = new


def make_in_maps(inputs):
    import ml_dtypes
    bf = ml_dtypes.bfloat16
    q = np.asarray(inputs["query"], np.float32)
    k = np.asarray(inputs["key"], np.float32)
    v = np.asarray(inputs["values"], np.float32)
    wqT = np.ascontiguousarray(np.asarray(inputs["Wq"], np.float32).T).astype(bf)
    wvT = np.ascontiguousarray(np.asarray(inputs["Wv"], np.float32).T).astype(bf)
    woT = np.ascontiguousarray(np.asarray(inputs["Wo"], np.float32).T).astype(bf)
    small = {
        "bq": np.ascontiguousarray(inputs["bq"], np.float32),
        "bv": np.ascontiguousarray(inputs["bv"], np.float32).astype(bf),
        "bo": np.ascontiguousarray(inputs["bo"], np.float32).astype(bf),
        "gam": np.ascontiguousarray(inputs["gammas"], np.float32),
        "lng": np.ascontiguousarray(inputs["ln_g"], np.float32),
        "lnb": np.ascontiguousarray(inputs["ln_b"], np.float32),
    }
    iota = (np.arange(T)[None, :] - np.arange(P)[:, None]).astype(np.float32)

    stripe_data = []
    for qtiles in (QT_A, QT_B):
        rows = np.concatenate([np.arange(g * P, (g + 1) * P) for g in qtiles])
        gcol = np.zeros((P, NQ), np.float32)
        zfix = np.zeros((P, NQ), np.float32)
        maskme = np.zeros((P, NQ, 2, P), np.float32)
        for jj, gi in enumerate(qtiles):
            gcol[:, jj] = -float(gi * P)
            if gi == 0:
                zfix[0, jj] = 1.0
            i_glob = gi * P + np.arange(P)[:, None]
            for tt in range(2):
                tpos = EXT[jj] - 2 + tt
                kk = tpos * P + np.arange(P)[None, :]
                maskme[:, jj, tt, :] = np.where(kk >= i_glob, NEG, 0.0)
        stripe_data.append(dict(
            rows=rows, gcol=gcol, zfix=zfix,
            maskme=maskme.reshape(P, NQ * 2 * P)))

    maps = []
    for c in range(8):
        sd = stripe_data[c // 4]
        b = c % 4
        rows = sd["rows"]
        m = dict(small)
        m["wqT"], m["wvT"], m["woT"] = wqT, wvT, woT
        m["xkT"] = np.ascontiguousarray(k[b].T).astype(bf)
        m["xvT"] = np.ascontiguousarray(v[b].T).astype(bf)
        m["xqT"] = np.ascontiguousarray(q[b].T[:, rows]).astype(bf)
        m["xq"] = np.ascontiguousarray(q[b][rows])
        m["iota"] = iota
        m["gcol"] = sd["gcol"]
        m["zfix"] = sd["zfix"]
        m["maskme"] = sd["maskme"].astype(bf)
        maps.append(m)
    return maps


class _Runner:
    def __init__(self):
        self.nc = build_program()
        self._fn = None

    def _make_fn(self, nc, devices):
        import jax
        from jax.sharding import Mesh, PartitionSpec
        from jax.experimental.shard_map import shard_map
        from concourse import bass2jax
        from concourse.bass2jax import _bass_exec_p, partition_id_tensor

        bass2jax.install_neuronx_cc_hook()
        partition_name = (nc.partition_id_tensor.name
                          if nc.partition_id_tensor else None)
        in_names, out_names, out_avals, zero_outs = [], [], [], []
        for alloc in nc.m.functions[0].allocations:
            if not isinstance(alloc, mybir.MemoryLocationSet):
                continue
            name = alloc.memorylocations[0].name
            if alloc.kind == "ExternalInput":
                if name != partition_name:
                    in_names.append(name)
            elif alloc.kind == "ExternalOutput":
                shape = tuple(alloc.tensor_shape)
                dtype = mybir.dt.np(alloc.dtype)
                out_names.append(name)
                out_avals.append(jax.core.ShapedArray(shape, dtype))
                zero_outs.append(np.zeros(shape, dtype))
        n_params = len(in_names)
        all_in = list(in_names) + list(out_names)
        if partition_name is not None:
            all_in.append(partition_name)

        def _body(*args):
            operands = list(args)
            if partition_name is not None:
                operands.append(partition_id_tensor())
            outs = _bass_exec_p.bind(
                *operands, out_avals=tuple(out_avals), in_names=tuple(all_in),
                out_names=tuple(out_names), lowering_input_output_aliases=(),
                sim_require_finite=True, sim_require_nnan=True, nc=nc)
            return tuple(outs)

        mesh = Mesh(np.asarray(devices), ("core",))
        n = n_params + len(out_names)
        fn = jax.jit(shard_map(_body, mesh=mesh,
                               in_specs=(PartitionSpec("core"),) * n,
                               out_specs=(PartitionSpec("core"),) * len(out_names),
                               check_rep=False),
                     keep_unused=True)
        return fn, in_names, out_names, zero_outs

    def fn(self):
        if self._fn is None:
            import jax
            self._fn = self._make_fn(self.nc, jax.devices()[:8])
        return self._fn

    def run(self, inputs):
        import jax
        fn, in_names, out_names, zero_outs = self.fn()
        maps = make_in_maps(inputs)
        args = [np.concatenate([np.asarray(m[nm]) for m in maps], axis=0)
                for nm in in_names]
        args += [np.zeros((8 * z.shape[0], *z.shape[1:]), z.dtype)
                 for z in zero_outs]
        outs = fn(*args)
        jax.block_until_ready(outs)
        y = np.asarray(outs[0]).reshape(8, TQ, D)
        out = np.empty((B, T, D), np.float32)
        for c in range(8):
            qtiles = (QT_A, QT_B)[c // 4]
            b = c % 4
            for jj, g in enumerate(qtiles):
                out[b, g * P:(g + 1) * P] = y[c, jj * P:(jj + 1) * P]
        return out


_runner = None


def kernel(**inputs) -> np.ndarray:
    global _runner
    if _runner is None:
        _runner = _Runner()
    return _runner.run(inputs)


# revision 21
# speedup vs baseline: 7523.6265x; 1.0121x over previous
"""Trainium2 Bass kernel for nn_DTransformer (sparse decay attention layer).

Single SPMD program on 8 cores: core c -> (stripe = c//4, batch = c%4).
Stripe A owns q-tiles {0,3,4,7}, stripe B {1,2,5,6} (of 8 tiles of 128
rows).  Both stripes process their j-th tile with a PADDED causal extent
EXT[j] = (2,4,6,8) k-tiles so the instruction stream is identical; the
true causal structure is injected via per-core input data (additive mask
tiles, iota offsets, zfix columns).

Math per (q-tile, head), S = (q@k^T)/sqrt(dk) with strict-causal mask:
  e    = exp(S)                       [ACT, accum -> Z]
  suf  = reversed-exclusive-cumsum(e) [DVE scan, neg-stride APs]
  sq   = suf^0.5 * sqrt(i-k)          [DVE stt pow+mult; sqrt((Z-cum)*pos)
                                       factorized]
  f    = exp(gz * sq), gz = -|gamma|/sqrt(Z)   [ACT, per-row scale]
  e2   = e^f = exp(S*f)               [DVE stt pow, accum -> Z2, bf16]
  m2   = rowmax(e2)                   [DVE tensor_mask_reduce]
  p    = e2 * min(Z2/m2,5)/Z2         [DVE tensor_scalar, bf16]
  out += p @ v                        [PE: transpose p + matmul]
Projections / output proj / residual / LayerNorm standard; weights and
activations are fed pre-transposed and bf16-cast from the host.
"""

import numpy as np

import concourse.bacc as bacc
import concourse.tile as tile
import concourse.bass as bass
from concourse import mybir
from concourse.masks import make_identity

P = 128
F32 = mybir.dt.float32
BF16 = mybir.dt.bfloat16
AF = mybir.ActivationFunctionType
ALU = mybir.AluOpType
NEG = -1.0e30

B, T, D, H = 4, 1024, 1024, 16
dk = D // H          # 64
ET = D // P          # 8 e-tiles
DT = D // P          # 8 contraction tiles
TT = T // P          # 8 t-tiles
NQ = 4               # q-tiles per core
TQ = NQ * P          # 512 q rows per core
EXT = [2, 4, 6, 8]   # padded k-extents (tiles) per q-slot, both stripes
QT_A = [0, 3, 4, 7]
QT_B = [1, 2, 5, 6]
EPS = 1e-5


def emit(tc, io):
    nc = tc.nc
    from contextlib import ExitStack
    st = ExitStack()

    cpool = st.enter_context(tc.tile_pool(name="consts", bufs=1))
    ppool = st.enter_context(tc.tile_pool(name="persist", bufs=1))

    # ---------------- constants ----------------
    ident = cpool.tile([P, P], F32)
    make_identity(nc, ident)
    ident_bf = cpool.tile([P, P], BF16)
    nc.vector.tensor_copy(out=ident_bf, in_=ident)

    ones1_bf = cpool.tile([1, P], BF16)
    nc.vector.memset(ones1_bf, 1.0)

    # biases per-partition per-etile: bq_pe[p, et] = bq[et*128+p]
    bq_pe = cpool.tile([P, ET], F32)
    nc.sync.dma_start(out=bq_pe, in_=bass.AP(
        tensor=io["bq"].tensor, offset=io["bq"].offset, ap=[[1, P], [P, ET]]))
    bq8 = cpool.tile([P, ET], F32)
    nc.vector.tensor_scalar_mul(bq8, bq_pe, 0.125)

    bv_bf = cpool.tile([1, D], BF16)
    nc.sync.dma_start(out=bv_bf, in_=io["bv"][None, :])
    bo_bf = cpool.tile([1, D], BF16)
    nc.sync.dma_start(out=bo_bf, in_=io["bo"][None, :])

    def bcast(src, cols, tag):
        t = cpool.tile([P, cols], F32, tag=tag)
        nc.sync.dma_start(out=t, in_=bass.AP(
            tensor=src.tensor, offset=src.offset, ap=[[0, P]] + src.ap))
        return t

    lng_bc = bcast(io["lng"], D, "lng_bc")
    lnb_bc = bcast(io["lnb"], D, "lnb_bc")
    gam_bc = bcast(io["gam"], H, "gam_bc")
    gneg = cpool.tile([P, H], F32)
    nc.scalar.activation(out=gneg, in_=gam_bc, func=AF.Abs)
    nc.vector.tensor_scalar_mul(gneg, gneg, -1.0)

    iota = cpool.tile([P, T], F32)        # k - qi
    nc.sync.dma_start(out=iota, in_=io["iota"])
    gcol = cpool.tile([P, NQ], F32)       # -gi*128 per slot
    nc.sync.dma_start(out=gcol, in_=io["gcol"])
    maskme = cpool.tile([P, NQ * 2 * P], BF16)   # additive mask, last 2 tiles/slot
    nc.sync.dma_start(out=maskme, in_=io["maskme"])

    eps_col = cpool.tile([P, 1], F32)
    nc.vector.memset(eps_col, EPS)
    eps30_col = cpool.tile([P, 1], F32)
    nc.vector.memset(eps30_col, 1e-30)

    # ---------------- persistent activations ----------------
    kT = ppool.tile([P, ET, T], BF16, tag="kT")       # (e, t) per e-tile
    qT = ppool.tile([P, ET, TQ], BF16, tag="qT")      # (e, q) prescaled 1/8
    vb = ppool.tile([P, TT, D], BF16, tag="vb")       # (t, e) natural
    concT = ppool.tile([P, ET, TQ], BF16, tag="concT")

    # ---------------- projections ----------------
    with tc.tile_pool(name="wx", bufs=1) as wx, \
         tc.tile_pool(name="ppsum", bufs=1, space="PSUM") as pp, \
         tc.tile_pool(name="vpsum", bufs=2, space="PSUM") as vp:
        wqT_sb = wx.tile([P, DT, D], BF16, tag="wqT")
        wvT_sb = wx.tile([P, DT, D], BF16, tag="wvT")
        xkT_sb = wx.tile([P, DT, T], BF16, tag="xkT")
        xvT_sb = wx.tile([P, DT, T], BF16, tag="xvT")
        xqT_sb = wx.tile([P, DT, TQ], BF16, tag="xqT")
        for dc in range(DT):
            r = slice(dc * P, (dc + 1) * P)
            nc.sync.dma_start(out=wqT_sb[:, dc, :], in_=io["wqT"][r, :])
            nc.sync.dma_start(out=xkT_sb[:, dc, :], in_=io["xkT"][r, :])
            nc.sync.dma_start(out=xqT_sb[:, dc, :], in_=io["xqT"][r, :])
            nc.sync.dma_start(out=wvT_sb[:, dc, :], in_=io["wvT"][r, :])
            nc.sync.dma_start(out=xvT_sb[:, dc, :], in_=io["xvT"][r, :])

        # k/q projections, et-major; shared stationary weight per (et, dc)
        for et in range(ET):
            es = slice(et * P, (et + 1) * P)
            kq = pp.tile([P, T], F32, tag="kq")
            qq = pp.tile([P, TQ], F32, tag="qq")
            for dc in range(DT):
                lw = wqT_sb[:, dc, es]
                nc.tensor.matmul(kq[:, 0:512], lw, xkT_sb[:, dc, 0:512],
                                 start=(dc == 0), stop=(dc == DT - 1))
                nc.tensor.matmul(kq[:, 512:1024], lw, xkT_sb[:, dc, 512:1024],
                                 start=(dc == 0), stop=(dc == DT - 1))
                nc.tensor.matmul(qq, lw, xqT_sb[:, dc, :],
                                 start=(dc == 0), stop=(dc == DT - 1))
            nc.scalar.activation(out=kT[:, et, :], in_=kq, func=AF.Identity,
                                 bias=bq_pe[:, et:et + 1])
            nc.scalar.activation(out=qT[:, et, :], in_=qq, func=AF.Identity,
                                 bias=bq8[:, et:et + 1], scale=0.125)

        # v projection, tt-major
        for tt in range(TT):
            ts_ = slice(tt * P, (tt + 1) * P)
            for fh in range(2):
                fs = slice(fh * 512, (fh + 1) * 512)
                vv = vp.tile([P, 512], F32, tag="vv")
                for dc in range(DT):
                    nc.tensor.matmul(vv, xvT_sb[:, dc, ts_], wvT_sb[:, dc, fs],
                                     start=(dc == 0), stop=False)
                nc.tensor.matmul(vv, ones1_bf, bv_bf[:, fs],
                                 start=False, stop=True)
                nc.scalar.activation(out=vb[:, tt, fs], in_=vv, func=AF.Copy)

    # ---------------- late weights ----------------
    lpool = st.enter_context(tc.tile_pool(name="late", bufs=1))
    woT_sb = lpool.tile([P, DT, D], BF16, tag="woT")
    qnat = lpool.tile([P, NQ, D], F32, tag="qnat")
    for dc in range(DT):
        nc.sync.dma_start(out=woT_sb[:, dc, :],
                          in_=io["woT"][dc * P:(dc + 1) * P, :])
    for j in range(NQ):
        nc.sync.dma_start(out=qnat[:, j, :], in_=io["xq"][j * P:(j + 1) * P, :])

    # ---------------- attention + output ----------------
    with tc.tile_pool(name="anp", bufs=2) as anp, \
         tc.tile_pool(name="abig", bufs=1) as abig, \
         tc.tile_pool(name="asm", bufs=2) as asm, \
         tc.tile_pool(name="spsum", bufs=3, space="PSUM") as sp, \
         tc.tile_pool(name="tpsum", bufs=1, space="PSUM") as tp_pool, \
         tc.tile_pool(name="otmp", bufs=2) as otmp, \
         tc.tile_pool(name="omini", bufs=2) as omini:

        for j in range(NQ):
            nkt = EXT[j]
            Lk = nkt * P
            qs = slice(j * P, (j + 1) * P)

            negp = anp.tile([P, T], F32, tag="negp")
            nc.vector.tensor_scalar(out=negp[:, :Lk], in0=iota[:, :Lk],
                                    scalar1=gcol[:, j:j + 1], scalar2=0.0,
                                    op0=ALU.add, op1=ALU.min)
            # negp broadcast over a head pair: [P, 2, Lk] with 0-stride
            negp2 = bass.AP(tensor=negp.tensor, offset=negp.offset,
                            ap=[negp.ap[0], [0, 2], [1, Lk]])

            for hp in range(H // 2):
                h0 = 2 * hp
                et = hp  # = h0 // 2
                Zp = asm.tile([P, 2], F32, tag="Zp")
                gz = asm.tile([P, 2], F32, tag="gz")
                Z2p = asm.tile([P, 2], F32, tag="Z2p")
                m2p = asm.tile([P, 2], F32, tag="m2p")
                Ss, e2s = [], []
                x_ = abig.tile([P, 2, T], F32, tag="x", bufs=3)
                # --- stage A per head: scores, exp(e) for the suffix scan ---
                for hh in range(2):
                    po = hh * dk
                    S = sp.tile([P, T], F32, tag="S")
                    Ss.append(S)
                    c0 = 0
                    while c0 < Lk - 256:
                        w = min(512, Lk - 256 - c0)
                        nc.tensor.matmul(S[:, c0:c0 + w],
                                         qT[po:po + dk, et, qs],
                                         kT[po:po + dk, et, c0:c0 + w],
                                         start=True, stop=True)
                        c0 += w
                    nc.tensor.matmul(S[:, Lk - 256:Lk],
                                     qT[po:po + dk, et, qs],
                                     kT[po:po + dk, et, Lk - 256:Lk],
                                     start=True, stop=False)
                    nc.tensor.matmul(S[:, Lk - 256:Lk], ident_bf,
                                     maskme[:, j * 256:(j + 1) * 256],
                                     start=False, stop=True)
                    e_ = abig.tile([P, T], F32, tag="e", bufs=5)
                    nc.scalar.activation(out=e_[:, :Lk], in_=S[:, :Lk],
                                         func=AF.Exp,
                                         accum_out=Zp[:, hh:hh + 1])
                    nc.vector.memset(x_[:, hh, Lk - 1:Lk], 0.0)
                    nc.vector.tensor_tensor_scan(
                        out=x_[:, hh, Lk - 2::-1], data0=e_[:, Lk - 1:0:-1],
                        data1=e_[:, Lk - 1:0:-1], initial=0.0,
                        op0=ALU.add, op1=ALU.bypass)
                # --- pair: x2 = suffix * (i-k); sqrt via ln/exp (same table
                # set as Exp); gz = -|gamma| / sqrt(Z + zfix) ---
                x2 = bass.AP(tensor=x_.tensor, offset=x_.offset,
                             ap=[x_.ap[0], [T, 2], [1, Lk]])
                nc.vector.scalar_tensor_tensor(
                    out=x2, in0=x2, scalar=-1.0,
                    in1=negp2, op0=ALU.mult, op1=ALU.mult)
                nc.scalar.activation(out=x2, in_=x2, func=AF.Ln)
                nc.scalar.activation(out=x2, in_=x2, func=AF.Exp, scale=0.5)
                nc.scalar.activation(out=gz, in_=Zp, func=AF.Ln,
                                     bias=eps30_col)
                nc.scalar.activation(out=gz, in_=gz, func=AF.Exp, scale=-0.5)
                nc.vector.tensor_tensor(out=gz, in0=gz,
                                        in1=gneg[:, h0:h0 + 2], op=ALU.mult)
                # --- stage B per head: f, S2 = S*f (psum), e2, rowmax ---
                for hh in range(2):
                    S = Ss[hh]
                    f_ = abig.tile([P, T], F32, tag="f", bufs=3)
                    nc.scalar.activation(out=f_[:, :Lk], in_=x_[:, hh, :Lk],
                                         func=AF.Exp, scale=gz[:, hh:hh + 1])
                    nc.vector.scalar_tensor_tensor(
                        out=S[:, :Lk], in0=f_[:, :Lk], scalar=1.0,
                        in1=S[:, :Lk], op0=ALU.mult, op1=ALU.mult)
                    e2 = abig.tile([P, T], BF16, tag="e2", bufs=4)
                    e2s.append(e2)
                    nc.scalar.activation(out=e2[:, :Lk], in_=S[:, :Lk],
                                         func=AF.Exp,
                                         accum_out=Z2p[:, hh:hh + 1])
                    nc.vector.tensor_reduce(
                        out=m2p[:, hh:hh + 1], in_=e2[:, :Lk],
                        axis=mybir.AxisListType.X, op=ALU.max)
                # --- pair tiny: cc = min(5/Z2', 1/m2') ---
                nc.vector.tensor_scalar_add(Z2p, Z2p, 1e-30)
                nc.vector.tensor_scalar_add(m2p, m2p, 1e-30)
                cc = asm.tile([P, 2], F32, tag="cc")
                rz2 = asm.tile([P, 2], F32, tag="rz2")
                nc.vector.reciprocal(out=rz2, in_=Z2p)
                rm2 = asm.tile([P, 2], F32, tag="rm2")
                nc.vector.reciprocal(out=rm2, in_=m2p)
                nc.vector.scalar_tensor_tensor(out=cc, in0=rz2, scalar=5.0,
                                               in1=rm2, op0=ALU.mult,
                                               op1=ALU.min)
                # --- stage C per head: rescale, transpose, PV ---
                opv = tp_pool.tile([P, 4 * P], F32, tag="opv")
                for hh in range(2):
                    h = h0 + hh
                    e2 = e2s[hh]
                    p_ = asm.tile([P, T], BF16, tag="p")
                    nc.vector.tensor_scalar_mul(p_[:, :Lk], e2[:, :Lk],
                                                cc[:, hh:hh + 1])
                    pT = asm.tile([P, T], BF16, tag="pT")
                    for g0 in range(0, nkt, 4):
                        gn = min(4, nkt - g0)
                        tp = tp_pool.tile([P, 4 * P], BF16, tag="tp")
                        for gg in range(gn):
                            kt = g0 + gg
                            nc.tensor.transpose(tp[:, gg * P:(gg + 1) * P],
                                                p_[:, kt * P:(kt + 1) * P],
                                                ident_bf)
                        nc.vector.tensor_copy(out=pT[:, g0 * P:(g0 + gn) * P],
                                              in_=tp[:, :gn * P])
                    for kt in range(nkt):
                        nc.tensor.matmul(opv[hh * dk:(hh + 1) * dk, 0:P],
                                         vb[:, kt, h * dk:(h + 1) * dk],
                                         pT[:, kt * P:(kt + 1) * P],
                                         start=(kt == 0), stop=(kt == nkt - 1))
                nc.scalar.activation(out=concT[:, et, qs], in_=opv[:, 0:P],
                                     func=AF.Copy)

            # ---- output projection + residual + layernorm for q-tile j ----
            xsb = otmp.tile([P, D], F32, tag="xsb")
            for fh in range(2):
                fs = slice(fh * 512, (fh + 1) * 512)
                ps = tp_pool.tile([P, 4 * P], F32, tag="opv")
                for et2 in range(ET):
                    nc.tensor.matmul(ps, concT[:, et2, qs],
                                     woT_sb[:, et2, fs],
                                     start=(et2 == 0), stop=False)
                nc.tensor.matmul(ps, ones1_bf, bo_bf[:, fs],
                                 start=False, stop=True)
                nc.vector.tensor_tensor(out=xsb[:, fs], in0=ps,
                                        in1=qnat[:, j, fs], op=ALU.add)
            stats = omini.tile([P, 2, 6], F32, tag="stats")
            for sg in range(2):
                nc.vector.bn_stats(out=stats[:, sg, :],
                                   in_=xsb[:, sg * 512:(sg + 1) * 512])
            mv = omini.tile([P, 2], F32, tag="mv")
            nc.vector.bn_aggr(out=mv, in_=stats)
            rstd = omini.tile([P, 1], F32, tag="rstd")
            nc.scalar.activation(out=rstd, in_=mv[:, 1:2], func=AF.Ln,
                                 bias=eps_col)
            nc.scalar.activation(out=rstd, in_=rstd, func=AF.Exp, scale=-0.5)
            nmr = omini.tile([P, 1], F32, tag="nmr")
            nc.vector.scalar_tensor_tensor(out=nmr, in0=mv[:, 0:1], scalar=-1.0,
                                           in1=rstd, op0=ALU.mult, op1=ALU.mult)
            ysb = otmp.tile([P, D], F32, tag="ysb")
            nc.scalar.activation(out=ysb, in_=xsb, func=AF.Identity,
                                 bias=nmr, scale=rstd)
            nc.vector.tensor_tensor(out=ysb, in0=ysb, in1=lng_bc, op=ALU.mult)
            nc.vector.tensor_tensor(out=ysb, in0=ysb, in1=lnb_bc, op=ALU.add)
            nc.sync.dma_start(out=io["y"][qs, :], in_=ysb)

    st.close()


# ------------------------------------------------------------------
# program build + host-side runner
# ------------------------------------------------------------------

def build_program():
    nc = bacc.Bacc("TRN2", target_bir_lowering=False, debug=False,
                   num_devices=8)
    io = {}

    def inp(name, shape, dt=F32):
        io[name] = nc.dram_tensor(name, shape, dt, kind="ExternalInput").ap()

    inp("wqT", [D, D], BF16)
    inp("wvT", [D, D], BF16)
    inp("woT", [D, D], BF16)
    inp("xkT", [D, T], BF16)
    inp("xvT", [D, T], BF16)
    inp("xqT", [D, TQ], BF16)
    inp("xq", [TQ, D])
    inp("bq", [D])
    inp("bv", [D], BF16)
    inp("bo", [D], BF16)
    inp("gam", [H])
    inp("lng", [D])
    inp("lnb", [D])
    inp("iota", [P, T])
    inp("gcol", [P, NQ])
    inp("maskme", [P, NQ * 2 * P], BF16)
    io["y"] = nc.dram_tensor("y", [TQ, D], F32, kind="ExternalOutput").ap()
    with tile.TileContext(nc) as tc:
        emit(tc, io)
    nc.compile()
    _unify_act_tables(nc)
    return nc


def _unify_act_tables(nc):
    """Retarget every ACT table load to natural_log_exp_and_others (which
    contains all functions this kernel uses: exp/ln/identity/copy/abs) and
    drop now-redundant consecutive loads.  The default chooser alternates
    exp_and_others <-> natural_log, costing ~2.7us per switch."""
    from concourse.hw_specs import get_activation_tables
    tables = get_activation_tables(nc.m.arch)
    names = list(tables.keys())
    target = names.index("natural_log_exp_and_others")
    allowed = tables["natural_log_exp_and_others"]
    used = set()
    for fn in nc.m.functions:
        for b in fn.blocks:
            for ins in b.instructions:
                if isinstance(ins, mybir.InstActivation):
                    used.add(ins.func)
    if not used <= allowed:
        return  # some function outside the combined set; keep default loads
    for fn in nc.m.functions:
        for b in fn.blocks:
            new = []
            cur = -1
            for ins in b.instructions:
                if (isinstance(ins, mybir.InstLoadActFuncSet)
                        and ins.sync_info is None):
                    ins.act_func_set_id = target
                    if cur == target:
                        continue
                    cur = target
                new.append(ins)
            b.instructions[:] = new


def make_in_maps(inputs):
    import ml_dtypes
    bf = ml_dtypes.bfloat16
    q = np.asarray(inputs["query"], np.float32)
    k = np.asarray(inputs["key"], np.float32)
    v = np.asarray(inputs["values"], np.float32)
    wqT = np.ascontiguousarray(np.asarray(inputs["Wq"], np.float32).T).astype(bf)
    wvT = np.ascontiguousarray(np.asarray(inputs["Wv"], np.float32).T).astype(bf)
    woT = np.ascontiguousarray(np.asarray(inputs["Wo"], np.float32).T).astype(bf)
    small = {
        "bq": np.ascontiguousarray(inputs["bq"], np.float32),
        "bv": np.ascontiguousarray(inputs["bv"], np.float32).astype(bf),
        "bo": np.ascontiguousarray(inputs["bo"], np.float32).astype(bf),
        "gam": np.ascontiguousarray(inputs["gammas"], np.float32),
        "lng": np.ascontiguousarray(inputs["ln_g"], np.float32),
        "lnb": np.ascontiguousarray(inputs["ln_b"], np.float32),
    }
    iota = (np.arange(T)[None, :] - np.arange(P)[:, None]).astype(np.float32)

    stripe_data = []
    for qtiles in (QT_A, QT_B):
        rows = np.concatenate([np.arange(g * P, (g + 1) * P) for g in qtiles])
        gcol = np.zeros((P, NQ), np.float32)
        maskme = np.zeros((P, NQ, 2, P), np.float32)
        for jj, gi in enumerate(qtiles):
            gcol[:, jj] = -float(gi * P)
            i_glob = gi * P + np.arange(P)[:, None]
            for tt in range(2):
                tpos = EXT[jj] - 2 + tt
                kk = tpos * P + np.arange(P)[None, :]
                maskme[:, jj, tt, :] = np.where(kk >= i_glob, NEG, 0.0)
        stripe_data.append(dict(
            rows=rows, gcol=gcol,
            maskme=maskme.reshape(P, NQ * 2 * P)))

    maps = []
    for c in range(8):
        sd = stripe_data[c // 4]
        b = c % 4
        rows = sd["rows"]
        m = dict(small)
        m["wqT"], m["wvT"], m["woT"] = wqT, wvT, woT
        m["xkT"] = np.ascontiguousarray(k[b].T).astype(bf)
        m["xvT"] = np.ascontiguousarray(v[b].T).astype(bf)
        m["xqT"] = np.ascontiguousarray(q[b].T[:, rows]).astype(bf)
        m["xq"] = np.ascontiguousarray(q[b][rows])
        m["iota"] = iota
        m["gcol"] = sd["gcol"]
        m["maskme"] = sd["maskme"].astype(bf)
        maps.append(m)
    return maps


class _Runner:
    def __init__(self):
        self.nc = build_program()
        self._fn = None

    def _make_fn(self, nc, devices):
        import jax
        from jax.sharding import Mesh, PartitionSpec
        from jax.experimental.shard_map import shard_map
        from concourse import bass2jax
        from concourse.bass2jax import _bass_exec_p, partition_id_tensor

        bass2jax.install_neuronx_cc_hook()
        partition_name = (nc.partition_id_tensor.name
                          if nc.partition_id_tensor else None)
        in_names, out_names, out_avals, zero_outs = [], [], [], []
        for alloc in nc.m.functions[0].allocations:
            if not isinstance(alloc, mybir.MemoryLocationSet):
                continue
            name = alloc.memorylocations[0].name
            if alloc.kind == "ExternalInput":
                if name != partition_name:
                    in_names.append(name)
            elif alloc.kind == "ExternalOutput":
                shape = tuple(alloc.tensor_shape)
                dtype = mybir.dt.np(alloc.dtype)
                out_names.append(name)
                out_avals.append(jax.core.ShapedArray(shape, dtype))
                zero_outs.append(np.zeros(shape, dtype))
        n_params = len(in_names)
        all_in = list(in_names) + list(out_names)
        if partition_name is not None:
            all_in.append(partition_name)

        def _body(*args):
            operands = list(args)
            if partition_name is not None:
                operands.append(partition_id_tensor())
            outs = _bass_exec_p.bind(
                *operands, out_avals=tuple(out_avals), in_names=tuple(all_in),
                out_names=tuple(out_names), lowering_input_output_aliases=(),
                sim_require_finite=True, sim_require_nnan=True, nc=nc)
            return tuple(outs)

        mesh = Mesh(np.asarray(devices), ("core",))
        n = n_params + len(out_names)
        fn = jax.jit(shard_map(_body, mesh=mesh,
                               in_specs=(PartitionSpec("core"),) * n,
                               out_specs=(PartitionSpec("core"),) * len(out_names),
                               check_rep=False),
                     keep_unused=True)
        return fn, in_names, out_names, zero_outs

    def fn(self):
        if self._fn is None:
            import jax
            self._fn = self._make_fn(self.nc, jax.devices()[:8])
        return self._fn

    def run(self, inputs):
        import jax
        fn, in_names, out_names, zero_outs = self.fn()
        maps = make_in_maps(inputs)
        args = [np.concatenate([np.asarray(m[nm]) for m in maps], axis=0)
                for nm in in_names]
        args += [np.zeros((8 * z.shape[0], *z.shape[1:]), z.dtype)
                 for z in zero_outs]
        outs = fn(*args)
        jax.block_until_ready(outs)
        y = np.asarray(outs[0]).reshape(8, TQ, D)
        out = np.empty((B, T, D), np.float32)
        for c in range(8):
            qtiles = (QT_A, QT_B)[c // 4]
            b = c % 4
            for jj, g in enumerate(qtiles):
                out[b, g * P:(g + 1) * P] = y[c, jj * P:(jj + 1) * P]
        return out


_runner = None


def kernel(**inputs) -> np.ndarray:
    global _runner
    if _runner is None:
        _runner = _Runner()
    return _runner.run(inputs)


# revision 22
# speedup vs baseline: 7531.2609x; 1.0010x over previous
"""Trainium2 Bass kernel for nn_DTransformer (sparse decay attention layer).

Single SPMD program on 8 cores: core c -> (stripe = c//4, batch = c%4).
Stripe A owns q-tiles {0,3,4,7}, stripe B {1,2,5,6} (of 8 tiles of 128
rows).  Both stripes process their j-th tile with a PADDED causal extent
EXT[j] = (2,4,6,8) k-tiles so the instruction stream is identical; the
true causal structure is injected via per-core input data (additive mask
tiles, iota offsets, zfix columns).

Math per (q-tile, head), S = (q@k^T)/sqrt(dk) with strict-causal mask:
  e    = exp(S)                       [ACT, accum -> Z]
  suf  = reversed-exclusive-cumsum(e) [DVE scan, neg-stride APs]
  sq   = suf^0.5 * sqrt(i-k)          [DVE stt pow+mult; sqrt((Z-cum)*pos)
                                       factorized]
  f    = exp(gz * sq), gz = -|gamma|/sqrt(Z)   [ACT, per-row scale]
  e2   = e^f = exp(S*f)               [DVE stt pow, accum -> Z2, bf16]
  m2   = rowmax(e2)                   [DVE tensor_mask_reduce]
  p    = e2 * min(Z2/m2,5)/Z2         [DVE tensor_scalar, bf16]
  out += p @ v                        [PE: transpose p + matmul]
Projections / output proj / residual / LayerNorm standard; weights and
activations are fed pre-transposed and bf16-cast from the host.
"""

import numpy as np

import concourse.bacc as bacc
import concourse.tile as tile
import concourse.bass as bass
from concourse import mybir
from concourse.masks import make_identity

P = 128
F32 = mybir.dt.float32
BF16 = mybir.dt.bfloat16
AF = mybir.ActivationFunctionType
ALU = mybir.AluOpType
NEG = -1.0e30

B, T, D, H = 4, 1024, 1024, 16
dk = D // H          # 64
ET = D // P          # 8 e-tiles
DT = D // P          # 8 contraction tiles
TT = T // P          # 8 t-tiles
NQ = 4               # q-tiles per core
TQ = NQ * P          # 512 q rows per core
EXT = [2, 4, 6, 8]   # padded k-extents (tiles) per q-slot, both stripes
QT_A = [0, 3, 4, 7]
QT_B = [1, 2, 5, 6]
EPS = 1e-5


def emit(tc, io):
    nc = tc.nc
    from contextlib import ExitStack
    st = ExitStack()

    cpool = st.enter_context(tc.tile_pool(name="consts", bufs=1))
    ppool = st.enter_context(tc.tile_pool(name="persist", bufs=1))

    # ---------------- constants ----------------
    ident = cpool.tile([P, P], F32)
    make_identity(nc, ident)
    ident_bf = cpool.tile([P, P], BF16)
    nc.vector.tensor_copy(out=ident_bf, in_=ident)

    ones1_bf = cpool.tile([1, P], BF16)
    nc.vector.memset(ones1_bf, 1.0)

    # biases per-partition per-etile: bq_pe[p, et] = bq[et*128+p]
    bq_pe = cpool.tile([P, ET], F32)
    nc.sync.dma_start(out=bq_pe, in_=bass.AP(
        tensor=io["bq"].tensor, offset=io["bq"].offset, ap=[[1, P], [P, ET]]))
    bq8 = cpool.tile([P, ET], F32)
    nc.vector.tensor_scalar_mul(bq8, bq_pe, 0.125)

    bv_bf = cpool.tile([1, D], BF16)
    nc.sync.dma_start(out=bv_bf, in_=io["bv"][None, :])
    bo_bf = cpool.tile([1, D], BF16)
    nc.sync.dma_start(out=bo_bf, in_=io["bo"][None, :])

    def bcast(src, cols, tag):
        t = cpool.tile([P, cols], F32, tag=tag)
        nc.sync.dma_start(out=t, in_=bass.AP(
            tensor=src.tensor, offset=src.offset, ap=[[0, P]] + src.ap))
        return t

    lng_bc = bcast(io["lng"], D, "lng_bc")
    lnb_bc = bcast(io["lnb"], D, "lnb_bc")
    gam_bc = bcast(io["gam"], H, "gam_bc")
    gneg = cpool.tile([P, H], F32)
    nc.scalar.activation(out=gneg, in_=gam_bc, func=AF.Abs)
    nc.vector.tensor_scalar_mul(gneg, gneg, -1.0)

    iota = cpool.tile([P, T], F32)        # k - qi
    nc.sync.dma_start(out=iota, in_=io["iota"])
    gcol = cpool.tile([P, NQ], F32)       # -gi*128 per slot
    nc.sync.dma_start(out=gcol, in_=io["gcol"])
    maskme = cpool.tile([P, NQ * 2 * P], BF16)   # additive mask, last 2 tiles/slot
    nc.sync.dma_start(out=maskme, in_=io["maskme"])

    eps_col = cpool.tile([P, 1], F32)
    nc.vector.memset(eps_col, EPS)
    eps30_col = cpool.tile([P, 1], F32)
    nc.vector.memset(eps30_col, 1e-30)

    # ---------------- persistent activations ----------------
    kT = ppool.tile([P, ET, T], BF16, tag="kT")       # (e, t) per e-tile
    qT = ppool.tile([P, ET, TQ], BF16, tag="qT")      # (e, q) prescaled 1/8
    vb = ppool.tile([P, TT, D], BF16, tag="vb")       # (t, e) natural
    concT = ppool.tile([P, ET, TQ], BF16, tag="concT")

    # ---------------- projections ----------------
    with tc.tile_pool(name="wx", bufs=1) as wx, \
         tc.tile_pool(name="ppsum", bufs=1, space="PSUM") as pp, \
         tc.tile_pool(name="vpsum", bufs=2, space="PSUM") as vp:
        wqT_sb = wx.tile([P, DT, D], BF16, tag="wqT")
        wvT_sb = wx.tile([P, DT, D], BF16, tag="wvT")
        xkT_sb = wx.tile([P, DT, T], BF16, tag="xkT")
        xvT_sb = wx.tile([P, DT, T], BF16, tag="xvT")
        xqT_sb = wx.tile([P, DT, TQ], BF16, tag="xqT")
        for dc in range(DT):
            r = slice(dc * P, (dc + 1) * P)
            nc.sync.dma_start(out=wqT_sb[:, dc, :], in_=io["wqT"][r, :])
            nc.sync.dma_start(out=xkT_sb[:, dc, :], in_=io["xkT"][r, :])
            nc.sync.dma_start(out=xqT_sb[:, dc, :], in_=io["xqT"][r, :])
            nc.sync.dma_start(out=wvT_sb[:, dc, :], in_=io["wvT"][r, :])
            nc.sync.dma_start(out=xvT_sb[:, dc, :], in_=io["xvT"][r, :])

        # k/q projections, et-major; shared stationary weight per (et, dc)
        for et in range(ET):
            es = slice(et * P, (et + 1) * P)
            kq = pp.tile([P, T], F32, tag="kq")
            qq = pp.tile([P, TQ], F32, tag="qq")
            for dc in range(DT):
                lw = wqT_sb[:, dc, es]
                nc.tensor.matmul(kq[:, 0:512], lw, xkT_sb[:, dc, 0:512],
                                 start=(dc == 0), stop=(dc == DT - 1))
                nc.tensor.matmul(kq[:, 512:1024], lw, xkT_sb[:, dc, 512:1024],
                                 start=(dc == 0), stop=(dc == DT - 1))
                nc.tensor.matmul(qq, lw, xqT_sb[:, dc, :],
                                 start=(dc == 0), stop=(dc == DT - 1))
            nc.scalar.activation(out=kT[:, et, :], in_=kq, func=AF.Identity,
                                 bias=bq_pe[:, et:et + 1])
            nc.scalar.activation(out=qT[:, et, :], in_=qq, func=AF.Identity,
                                 bias=bq8[:, et:et + 1], scale=0.125)

        # v projection, tt-major
        for tt in range(TT):
            ts_ = slice(tt * P, (tt + 1) * P)
            for fh in range(2):
                fs = slice(fh * 512, (fh + 1) * 512)
                vv = vp.tile([P, 512], F32, tag="vv")
                for dc in range(DT):
                    nc.tensor.matmul(vv, xvT_sb[:, dc, ts_], wvT_sb[:, dc, fs],
                                     start=(dc == 0), stop=False)
                nc.tensor.matmul(vv, ones1_bf, bv_bf[:, fs],
                                 start=False, stop=True)
                nc.scalar.activation(out=vb[:, tt, fs], in_=vv, func=AF.Copy)

    # ---------------- late weights ----------------
    lpool = st.enter_context(tc.tile_pool(name="late", bufs=1))
    woT_sb = lpool.tile([P, DT, D], BF16, tag="woT")
    qnat = lpool.tile([P, NQ, D], F32, tag="qnat")
    for dc in range(DT):
        nc.sync.dma_start(out=woT_sb[:, dc, :],
                          in_=io["woT"][dc * P:(dc + 1) * P, :])
    for j in range(NQ):
        nc.sync.dma_start(out=qnat[:, j, :], in_=io["xq"][j * P:(j + 1) * P, :])

    # ---------------- attention + output ----------------
    with tc.tile_pool(name="anp", bufs=2) as anp, \
         tc.tile_pool(name="abig", bufs=1) as abig, \
         tc.tile_pool(name="asm", bufs=2) as asm, \
         tc.tile_pool(name="spsum", bufs=3, space="PSUM") as sp, \
         tc.tile_pool(name="tpsum", bufs=1, space="PSUM") as tp_pool, \
         tc.tile_pool(name="otmp", bufs=2) as otmp, \
         tc.tile_pool(name="omini", bufs=2) as omini:

        for j in range(NQ):
            nkt = EXT[j]
            Lk = nkt * P
            qs = slice(j * P, (j + 1) * P)

            negp = anp.tile([P, T], F32, tag="negp")
            nc.vector.tensor_scalar(out=negp[:, :Lk], in0=iota[:, :Lk],
                                    scalar1=gcol[:, j:j + 1], scalar2=0.0,
                                    op0=ALU.add, op1=ALU.min)
            # negp broadcast over a head pair: [P, 2, Lk] with 0-stride
            negp2 = bass.AP(tensor=negp.tensor, offset=negp.offset,
                            ap=[negp.ap[0], [0, 2], [1, Lk]])

            for hp in range(H // 2):
                h0 = 2 * hp
                et = hp  # = h0 // 2
                Zp = asm.tile([P, 2], F32, tag="Zp")
                gz = asm.tile([P, 2], F32, tag="gz")
                Z2p = asm.tile([P, 2], F32, tag="Z2p")
                m2p = asm.tile([P, 2], F32, tag="m2p")
                Ss, e2s = [], []
                x_ = abig.tile([P, 2, T], F32, tag="x", bufs=3)
                # --- stage A per head: scores, exp(e) for the suffix scan ---
                for hh in range(2):
                    po = hh * dk
                    S = sp.tile([P, T], F32, tag="S")
                    Ss.append(S)
                    c0 = 0
                    while c0 < Lk - 256:
                        w = min(512, Lk - 256 - c0)
                        nc.tensor.matmul(S[:, c0:c0 + w],
                                         qT[po:po + dk, et, qs],
                                         kT[po:po + dk, et, c0:c0 + w],
                                         start=True, stop=True)
                        c0 += w
                    nc.tensor.matmul(S[:, Lk - 256:Lk],
                                     qT[po:po + dk, et, qs],
                                     kT[po:po + dk, et, Lk - 256:Lk],
                                     start=True, stop=False)
                    nc.tensor.matmul(S[:, Lk - 256:Lk], ident_bf,
                                     maskme[:, j * 256:(j + 1) * 256],
                                     start=False, stop=True)
                    e_ = abig.tile([P, T], F32, tag="e", bufs=5)
                    nc.scalar.activation(out=e_[:, :Lk], in_=S[:, :Lk],
                                         func=AF.Exp,
                                         accum_out=Zp[:, hh:hh + 1])
                    nc.vector.memset(x_[:, hh, Lk - 1:Lk], 0.0)
                    nc.vector.tensor_tensor_scan(
                        out=x_[:, hh, Lk - 2::-1], data0=e_[:, Lk - 1:0:-1],
                        data1=e_[:, Lk - 1:0:-1], initial=0.0,
                        op0=ALU.add, op1=ALU.bypass)
                # --- pair: x2 = suffix * (i-k); sqrt via ln/exp (same table
                # set as Exp); gz = -|gamma| / sqrt(Z + zfix) ---
                x2 = bass.AP(tensor=x_.tensor, offset=x_.offset,
                             ap=[x_.ap[0], [T, 2], [1, Lk]])
                nc.vector.scalar_tensor_tensor(
                    out=x2, in0=x2, scalar=-1.0,
                    in1=negp2, op0=ALU.mult, op1=ALU.mult)
                nc.scalar.activation(out=x2, in_=x2, func=AF.Ln)
                nc.scalar.activation(out=x2, in_=x2, func=AF.Exp, scale=0.5)
                nc.scalar.activation(out=gz, in_=Zp, func=AF.Ln,
                                     bias=eps30_col)
                nc.scalar.activation(out=gz, in_=gz, func=AF.Exp, scale=-0.5)
                nc.vector.tensor_tensor(out=gz, in0=gz,
                                        in1=gneg[:, h0:h0 + 2], op=ALU.mult)
                # --- stage B per head: f, S2 = S*f (psum), e2, rowmax ---
                for hh in range(2):
                    S = Ss[hh]
                    f_ = abig.tile([P, T], F32, tag="f", bufs=3)
                    nc.scalar.activation(out=f_[:, :Lk], in_=x_[:, hh, :Lk],
                                         func=AF.Exp, scale=gz[:, hh:hh + 1])
                    nc.vector.scalar_tensor_tensor(
                        out=S[:, :Lk], in0=f_[:, :Lk], scalar=1.0,
                        in1=S[:, :Lk], op0=ALU.mult, op1=ALU.mult)
                    e2 = abig.tile([P, T], BF16, tag="e2", bufs=4)
                    e2s.append(e2)
                    nc.scalar.activation(out=e2[:, :Lk], in_=S[:, :Lk],
                                         func=AF.Exp,
                                         accum_out=Z2p[:, hh:hh + 1])
                    nc.vector.tensor_reduce(
                        out=m2p[:, hh:hh + 1], in_=e2[:, :Lk],
                        axis=mybir.AxisListType.X, op=ALU.max)
                # --- pair tiny: cc = min(5/Z2', 1/m2') ---
                nc.vector.tensor_scalar_add(Z2p, Z2p, 1e-30)
                nc.vector.tensor_scalar_add(m2p, m2p, 1e-30)
                cc = asm.tile([P, 2], F32, tag="cc")
                rz2 = asm.tile([P, 2], F32, tag="rz2")
                nc.vector.reciprocal(out=rz2, in_=Z2p)
                rm2 = asm.tile([P, 2], F32, tag="rm2")
                nc.vector.reciprocal(out=rm2, in_=m2p)
                nc.vector.scalar_tensor_tensor(out=cc, in0=rz2, scalar=5.0,
                                               in1=rm2, op0=ALU.mult,
                                               op1=ALU.min)
                # --- stage C per head: rescale, transpose, PV ---
                opv = tp_pool.tile([P, 4 * P], F32, tag="opv")
                for hh in range(2):
                    h = h0 + hh
                    e2 = e2s[hh]
                    p_ = asm.tile([P, T], BF16, tag="p")
                    nc.vector.tensor_scalar_mul(p_[:, :Lk], e2[:, :Lk],
                                                cc[:, hh:hh + 1])
                    pT = asm.tile([P, TT, P], BF16, tag="pT")
                    nc.sync.dma_start_transpose(out=pT[:, :nkt, :],
                                                in_=p_[:, :Lk])
                    for kt in range(nkt):
                        nc.tensor.matmul(opv[hh * dk:(hh + 1) * dk, 0:P],
                                         vb[:, kt, h * dk:(h + 1) * dk],
                                         pT[:, kt, :],
                                         start=(kt == 0), stop=(kt == nkt - 1))
                nc.scalar.activation(out=concT[:, et, qs], in_=opv[:, 0:P],
                                     func=AF.Copy)

            # ---- output projection + residual + layernorm for q-tile j ----
            xsb = otmp.tile([P, D], F32, tag="xsb")
            for fh in range(2):
                fs = slice(fh * 512, (fh + 1) * 512)
                ps = tp_pool.tile([P, 4 * P], F32, tag="opv")
                for et2 in range(ET):
                    nc.tensor.matmul(ps, concT[:, et2, qs],
                                     woT_sb[:, et2, fs],
                                     start=(et2 == 0), stop=False)
                nc.tensor.matmul(ps, ones1_bf, bo_bf[:, fs],
                                 start=False, stop=True)
                nc.vector.tensor_tensor(out=xsb[:, fs], in0=ps,
                                        in1=qnat[:, j, fs], op=ALU.add)
            stats = omini.tile([P, 2, 6], F32, tag="stats")
            for sg in range(2):
                nc.vector.bn_stats(out=stats[:, sg, :],
                                   in_=xsb[:, sg * 512:(sg + 1) * 512])
            mv = omini.tile([P, 2], F32, tag="mv")
            nc.vector.bn_aggr(out=mv, in_=stats)
            rstd = omini.tile([P, 1], F32, tag="rstd")
            nc.scalar.activation(out=rstd, in_=mv[:, 1:2], func=AF.Ln,
                                 bias=eps_col)
            nc.scalar.activation(out=rstd, in_=rstd, func=AF.Exp, scale=-0.5)
            nmr = omini.tile([P, 1], F32, tag="nmr")
            nc.vector.scalar_tensor_tensor(out=nmr, in0=mv[:, 0:1], scalar=-1.0,
                                           in1=rstd, op0=ALU.mult, op1=ALU.mult)
            ysb = otmp.tile([P, D], F32, tag="ysb")
            nc.scalar.activation(out=ysb, in_=xsb, func=AF.Identity,
                                 bias=nmr, scale=rstd)
            nc.vector.tensor_tensor(out=ysb, in0=ysb, in1=lng_bc, op=ALU.mult)
            nc.vector.tensor_tensor(out=ysb, in0=ysb, in1=lnb_bc, op=ALU.add)
            nc.sync.dma_start(out=io["y"][qs, :], in_=ysb)

    st.close()


# ------------------------------------------------------------------
# program build + host-side runner
# ------------------------------------------------------------------

def build_program():
    nc = bacc.Bacc("TRN2", target_bir_lowering=False, debug=False,
                   num_devices=8)
    io = {}

    def inp(name, shape, dt=F32):
        io[name] = nc.dram_tensor(name, shape, dt, kind="ExternalInput").ap()

    inp("wqT", [D, D], BF16)
    inp("wvT", [D, D], BF16)
    inp("woT", [D, D], BF16)
    inp("xkT", [D, T], BF16)
    inp("xvT", [D, T], BF16)
    inp("xqT", [D, TQ], BF16)
    inp("xq", [TQ, D])
    inp("bq", [D])
    inp("bv", [D], BF16)
    inp("bo", [D], BF16)
    inp("gam", [H])
    inp("lng", [D])
    inp("lnb", [D])
    inp("iota", [P, T])
    inp("gcol", [P, NQ])
    inp("maskme", [P, NQ * 2 * P], BF16)
    io["y"] = nc.dram_tensor("y", [TQ, D], F32, kind="ExternalOutput").ap()
    with tile.TileContext(nc) as tc:
        emit(tc, io)
    nc.compile()
    _unify_act_tables(nc)
    return nc


def _unify_act_tables(nc):
    """Retarget every ACT table load to natural_log_exp_and_others (which
    contains all functions this kernel uses: exp/ln/identity/copy/abs) and
    drop now-redundant consecutive loads.  The default chooser alternates
    exp_and_others <-> natural_log, costing ~2.7us per switch."""
    from concourse.hw_specs import get_activation_tables
    tables = get_activation_tables(nc.m.arch)
    names = list(tables.keys())
    target = names.index("natural_log_exp_and_others")
    allowed = tables["natural_log_exp_and_others"]
    used = set()
    for fn in nc.m.functions:
        for b in fn.blocks:
            for ins in b.instructions:
                if isinstance(ins, mybir.InstActivation):
                    used.add(ins.func)
    if not used <= allowed:
        return  # some function outside the combined set; keep default loads
    for fn in nc.m.functions:
        for b in fn.blocks:
            new = []
            cur = -1
            for ins in b.instructions:
                if (isinstance(ins, mybir.InstLoadActFuncSet)
                        and ins.sync_info is None):
                    ins.act_func_set_id = target
                    if cur == target:
                        continue
                    cur = target
                new.append(ins)
            b.instructions[:] = new


def make_in_maps(inputs):
    import ml_dtypes
    bf = ml_dtypes.bfloat16
    q = np.asarray(inputs["query"], np.float32)
    k = np.asarray(inputs["key"], np.float32)
    v = np.asarray(inputs["values"], np.float32)
    wqT = np.ascontiguousarray(np.asarray(inputs["Wq"], np.float32).T).astype(bf)
    wvT = np.ascontiguousarray(np.asarray(inputs["Wv"], np.float32).T).astype(bf)
    woT = np.ascontiguousarray(np.asarray(inputs["Wo"], np.float32).T).astype(bf)
    small = {
        "bq": np.ascontiguousarray(inputs["bq"], np.float32),
        "bv": np.ascontiguousarray(inputs["bv"], np.float32).astype(bf),
        "bo": np.ascontiguousarray(inputs["bo"], np.float32).astype(bf),
        "gam": np.ascontiguousarray(inputs["gammas"], np.float32),
        "lng": np.ascontiguousarray(inputs["ln_g"], np.float32),
        "lnb": np.ascontiguousarray(inputs["ln_b"], np.float32),
    }
    iota = (np.arange(T)[None, :] - np.arange(P)[:, None]).astype(np.float32)

    stripe_data = []
    for qtiles in (QT_A, QT_B):
        rows = np.concatenate([np.arange(g * P, (g + 1) * P) for g in qtiles])
        gcol = np.zeros((P, NQ), np.float32)
        maskme = np.zeros((P, NQ, 2, P), np.float32)
        for jj, gi in enumerate(qtiles):
            gcol[:, jj] = -float(gi * P)
            i_glob = gi * P + np.arange(P)[:, None]
            for tt in range(2):
                tpos = EXT[jj] - 2 + tt
                kk = tpos * P + np.arange(P)[None, :]
                maskme[:, jj, tt, :] = np.where(kk >= i_glob, NEG, 0.0)
        stripe_data.append(dict(
            rows=rows, gcol=gcol,
            maskme=maskme.reshape(P, NQ * 2 * P)))

    maps = []
    for c in range(8):
        sd = stripe_data[c // 4]
        b = c % 4
        rows = sd["rows"]
        m = dict(small)
        m["wqT"], m["wvT"], m["woT"] = wqT, wvT, woT
        m["xkT"] = np.ascontiguousarray(k[b].T).astype(bf)
        m["xvT"] = np.ascontiguousarray(v[b].T).astype(bf)
        m["xqT"] = np.ascontiguousarray(q[b].T[:, rows]).astype(bf)
        m["xq"] = np.ascontiguousarray(q[b][rows])
        m["iota"] = iota
        m["gcol"] = sd["gcol"]
        m["maskme"] = sd["maskme"].astype(bf)
        maps.append(m)
    return maps


class _Runner:
    def __init__(self):
        self.nc = build_program()
        self._fn = None

    def _make_fn(self, nc, devices):
        import jax
        from jax.sharding import Mesh, PartitionSpec
        from jax.experimental.shard_map import shard_map
        from concourse import bass2jax
        from concourse.bass2jax import _bass_exec_p, partition_id_tensor

        bass2jax.install_neuronx_cc_hook()
        partition_name = (nc.partition_id_tensor.name
                          if nc.partition_id_tensor else None)
        in_names, out_names, out_avals, zero_outs = [], [], [], []
        for alloc in nc.m.functions[0].allocations:
            if not isinstance(alloc, mybir.MemoryLocationSet):
                continue
            name = alloc.memorylocations[0].name
            if alloc.kind == "ExternalInput":
                if name != partition_name:
                    in_names.append(name)
            elif alloc.kind == "ExternalOutput":
                shape = tuple(alloc.tensor_shape)
                dtype = mybir.dt.np(alloc.dtype)
                out_names.append(name)
                out_avals.append(jax.core.ShapedArray(shape, dtype))
                zero_outs.append(np.zeros(shape, dtype))
        n_params = len(in_names)
        all_in = list(in_names) + list(out_names)
        if partition_name is not None:
            all_in.append(partition_name)

        def _body(*args):
            operands = list(args)
            if partition_name is not None:
                operands.append(partition_id_tensor())
            outs = _bass_exec_p.bind(
                *operands, out_avals=tuple(out_avals), in_names=tuple(all_in),
                out_names=tuple(out_names), lowering_input_output_aliases=(),
                sim_require_finite=True, sim_require_nnan=True, nc=nc)
            return tuple(outs)

        mesh = Mesh(np.asarray(devices), ("core",))
        n = n_params + len(out_names)
        fn = jax.jit(shard_map(_body, mesh=mesh,
                               in_specs=(PartitionSpec("core"),) * n,
                               out_specs=(PartitionSpec("core"),) * len(out_names),
                               check_rep=False),
                     keep_unused=True)
        return fn, in_names, out_names, zero_outs

    def fn(self):
        if self._fn is None:
            import jax
            self._fn = self._make_fn(self.nc, jax.devices()[:8])
        return self._fn

    def run(self, inputs):
        import jax
        fn, in_names, out_names, zero_outs = self.fn()
        maps = make_in_maps(inputs)
        args = [np.concatenate([np.asarray(m[nm]) for m in maps], axis=0)
                for nm in in_names]
        args += [np.zeros((8 * z.shape[0], *z.shape[1:]), z.dtype)
                 for z in zero_outs]
        outs = fn(*args)
        jax.block_until_ready(outs)
        y = np.asarray(outs[0]).reshape(8, TQ, D)
        out = np.empty((B, T, D), np.float32)
        for c in range(8):
            qtiles = (QT_A, QT_B)[c // 4]
            b = c % 4
            for jj, g in enumerate(qtiles):
                out[b, g * P:(g + 1) * P] = y[c, jj * P:(jj + 1) * P]
        return out


_runner = None


def kernel(**inputs) -> np.ndarray:
    global _runner
    if _runner is None:
        _runner = _Runner()
    return _runner.run(inputs)


# revision 24
# speedup vs baseline: 7542.1561x; 1.0014x over previous
"""Trainium2 Bass kernel for nn_DTransformer (sparse decay attention layer).

Single SPMD program on 8 cores: core c -> (stripe = c//4, batch = c%4).
Stripe A owns q-tiles {0,3,4,7}, stripe B {1,2,5,6} (of 8 tiles of 128
rows).  Both stripes process their j-th tile with a PADDED causal extent
EXT[j] = (2,4,6,8) k-tiles so the instruction stream is identical; the
true causal structure is injected via per-core input data (additive mask
tiles, iota offsets, zfix columns).

Math per (q-tile, head), S = (q@k^T)/sqrt(dk) with strict-causal mask:
  e    = exp(S)                       [ACT, accum -> Z]
  suf  = reversed-exclusive-cumsum(e) [DVE scan, neg-stride APs]
  sq   = suf^0.5 * sqrt(i-k)          [DVE stt pow+mult; sqrt((Z-cum)*pos)
                                       factorized]
  f    = exp(gz * sq), gz = -|gamma|/sqrt(Z)   [ACT, per-row scale]
  e2   = e^f = exp(S*f)               [DVE stt pow, accum -> Z2, bf16]
  m2   = rowmax(e2)                   [DVE tensor_mask_reduce]
  p    = e2 * min(Z2/m2,5)/Z2         [DVE tensor_scalar, bf16]
  out += p @ v                        [PE: transpose p + matmul]
Projections / output proj / residual / LayerNorm standard; weights and
activations are fed pre-transposed and bf16-cast from the host.
"""

import numpy as np

import concourse.bacc as bacc
import concourse.tile as tile
import concourse.bass as bass
from concourse import mybir
from concourse.masks import make_identity

P = 128
F32 = mybir.dt.float32
BF16 = mybir.dt.bfloat16
AF = mybir.ActivationFunctionType
ALU = mybir.AluOpType
NEG = -1.0e30

B, T, D, H = 4, 1024, 1024, 16
dk = D // H          # 64
ET = D // P          # 8 e-tiles
DT = D // P          # 8 contraction tiles
TT = T // P          # 8 t-tiles
NQ = 4               # q-tiles per core
TQ = NQ * P          # 512 q rows per core
EXT = [2, 4, 6, 8]   # padded k-extents (tiles) per q-slot, both stripes
QT_A = [0, 3, 4, 7]
QT_B = [1, 2, 5, 6]
EPS = 1e-5


def emit(tc, io):
    nc = tc.nc
    from contextlib import ExitStack
    st = ExitStack()

    cpool = st.enter_context(tc.tile_pool(name="consts", bufs=1))
    ppool = st.enter_context(tc.tile_pool(name="persist", bufs=1))

    # ---------------- constants ----------------
    ident = cpool.tile([P, P], F32)
    make_identity(nc, ident)
    ident_bf = cpool.tile([P, P], BF16)
    nc.vector.tensor_copy(out=ident_bf, in_=ident)

    ones1_bf = cpool.tile([1, P], BF16)
    nc.vector.memset(ones1_bf, 1.0)

    # biases per-partition per-etile: bq_pe[p, et] = bq[et*128+p]
    bq_pe = cpool.tile([P, ET], F32)
    nc.sync.dma_start(out=bq_pe, in_=bass.AP(
        tensor=io["bq"].tensor, offset=io["bq"].offset, ap=[[1, P], [P, ET]]))
    bq8 = cpool.tile([P, ET], F32)
    nc.vector.tensor_scalar_mul(bq8, bq_pe, 0.125)

    bv_bf = cpool.tile([1, D], BF16)
    nc.sync.dma_start(out=bv_bf, in_=io["bv"][None, :])
    bo_bf = cpool.tile([1, D], BF16)
    nc.sync.dma_start(out=bo_bf, in_=io["bo"][None, :])

    def bcast(src, cols, tag):
        t = cpool.tile([P, cols], F32, tag=tag)
        nc.sync.dma_start(out=t, in_=bass.AP(
            tensor=src.tensor, offset=src.offset, ap=[[0, P]] + src.ap))
        return t

    lng_bc = bcast(io["lng"], D, "lng_bc")
    lnb_bc = bcast(io["lnb"], D, "lnb_bc")
    gam_bc = bcast(io["gam"], H, "gam_bc")
    gneg = cpool.tile([P, H], F32)
    nc.scalar.activation(out=gneg, in_=gam_bc, func=AF.Abs)
    nc.vector.tensor_scalar_mul(gneg, gneg, -1.0)

    iota = cpool.tile([P, T], F32)        # k - qi
    nc.sync.dma_start(out=iota, in_=io["iota"])
    gcol = cpool.tile([P, NQ], F32)       # -gi*128 per slot
    nc.sync.dma_start(out=gcol, in_=io["gcol"])
    maskme = cpool.tile([P, NQ * 2 * P], BF16)   # additive mask, last 2 tiles/slot
    nc.sync.dma_start(out=maskme, in_=io["maskme"])

    eps_col = cpool.tile([P, 1], F32)
    nc.vector.memset(eps_col, EPS)
    eps30_col = cpool.tile([P, 1], F32)
    nc.vector.memset(eps30_col, 1e-30)

    # ---------------- persistent activations ----------------
    kT = ppool.tile([P, ET, T], BF16, tag="kT")       # (e, t) per e-tile
    qT = ppool.tile([P, ET, TQ], BF16, tag="qT")      # (e, q) prescaled 1/8
    vb = ppool.tile([P, TT, D], BF16, tag="vb")       # (t, e) natural
    concT = ppool.tile([P, ET, TQ], BF16, tag="concT")

    # ---------------- projections ----------------
    with tc.tile_pool(name="wx", bufs=1) as wx, \
         tc.tile_pool(name="ppsum", bufs=1, space="PSUM") as pp, \
         tc.tile_pool(name="vpsum", bufs=2, space="PSUM") as vp:
        wqT_sb = wx.tile([P, DT, D], BF16, tag="wqT")
        wvT_sb = wx.tile([P, DT, D], BF16, tag="wvT")
        xkT_sb = wx.tile([P, DT, T], BF16, tag="xkT")
        xvT_sb = wx.tile([P, DT, T], BF16, tag="xvT")
        xqT_sb = wx.tile([P, DT, TQ], BF16, tag="xqT")
        for dc in range(DT):
            r = slice(dc * P, (dc + 1) * P)
            nc.sync.dma_start(out=wqT_sb[:, dc, :], in_=io["wqT"][r, :])
            nc.sync.dma_start(out=xkT_sb[:, dc, :], in_=io["xkT"][r, :])
            nc.sync.dma_start(out=xqT_sb[:, dc, :], in_=io["xqT"][r, :])
            nc.sync.dma_start(out=wvT_sb[:, dc, :], in_=io["wvT"][r, :])
            nc.sync.dma_start(out=xvT_sb[:, dc, :], in_=io["xvT"][r, :])

        # k/q projections, et-major; shared stationary weight per (et, dc)
        for et in range(ET):
            es = slice(et * P, (et + 1) * P)
            kq = pp.tile([P, T], F32, tag="kq")
            qq = pp.tile([P, TQ], F32, tag="qq")
            for dc in range(DT):
                lw = wqT_sb[:, dc, es]
                nc.tensor.matmul(kq[:, 0:512], lw, xkT_sb[:, dc, 0:512],
                                 start=(dc == 0), stop=(dc == DT - 1))
                nc.tensor.matmul(kq[:, 512:1024], lw, xkT_sb[:, dc, 512:1024],
                                 start=(dc == 0), stop=(dc == DT - 1))
                nc.tensor.matmul(qq, lw, xqT_sb[:, dc, :],
                                 start=(dc == 0), stop=(dc == DT - 1))
            nc.scalar.activation(out=kT[:, et, :], in_=kq, func=AF.Identity,
                                 bias=bq_pe[:, et:et + 1])
            nc.scalar.activation(out=qT[:, et, :], in_=qq, func=AF.Identity,
                                 bias=bq8[:, et:et + 1], scale=0.125)

        # v projection, tt-major
        for tt in range(TT):
            ts_ = slice(tt * P, (tt + 1) * P)
            for fh in range(2):
                fs = slice(fh * 512, (fh + 1) * 512)
                vv = vp.tile([P, 512], F32, tag="vv")
                for dc in range(DT):
                    nc.tensor.matmul(vv, xvT_sb[:, dc, ts_], wvT_sb[:, dc, fs],
                                     start=(dc == 0), stop=False)
                nc.tensor.matmul(vv, ones1_bf, bv_bf[:, fs],
                                 start=False, stop=True)
                nc.scalar.activation(out=vb[:, tt, fs], in_=vv, func=AF.Copy)

    # ---------------- late weights ----------------
    lpool = st.enter_context(tc.tile_pool(name="late", bufs=1))
    woT_sb = lpool.tile([P, DT, D], BF16, tag="woT")
    qnat = lpool.tile([P, NQ, D], F32, tag="qnat")
    for dc in range(DT):
        nc.sync.dma_start(out=woT_sb[:, dc, :],
                          in_=io["woT"][dc * P:(dc + 1) * P, :])
    for j in range(NQ):
        nc.sync.dma_start(out=qnat[:, j, :], in_=io["xq"][j * P:(j + 1) * P, :])

    # ---------------- attention + output ----------------
    with tc.tile_pool(name="anp", bufs=2) as anp, \
         tc.tile_pool(name="abig", bufs=1) as abig, \
         tc.tile_pool(name="asm", bufs=2) as asm, \
         tc.tile_pool(name="spsum", bufs=3, space="PSUM") as sp, \
         tc.tile_pool(name="tpsum", bufs=1, space="PSUM") as tp_pool, \
         tc.tile_pool(name="otmp", bufs=2) as otmp, \
         tc.tile_pool(name="omini", bufs=2) as omini:

        for j in range(NQ):
            nkt = EXT[j]
            Lk = nkt * P
            qs = slice(j * P, (j + 1) * P)

            negp = anp.tile([P, T], F32, tag="negp")
            nc.vector.tensor_scalar(out=negp[:, :Lk], in0=iota[:, :Lk],
                                    scalar1=gcol[:, j:j + 1], scalar2=0.0,
                                    op0=ALU.add, op1=ALU.min)
            # negp broadcast over a head pair: [P, 2, Lk] with 0-stride
            negp2 = bass.AP(tensor=negp.tensor, offset=negp.offset,
                            ap=[negp.ap[0], [0, 2], [1, Lk]])

            for hp in range(H // 2):
                h0 = 2 * hp
                et = hp  # = h0 // 2
                Zp = asm.tile([P, 2], F32, tag="Zp")
                gz = asm.tile([P, 2], F32, tag="gz")
                Z2p = asm.tile([P, 2], F32, tag="Z2p")
                m2p = asm.tile([P, 2], F32, tag="m2p")
                Ss, e2s = [], []
                x_ = abig.tile([P, 2, T], F32, tag="x", bufs=3)
                # --- stage A per head: scores, exp(e) for the suffix scan ---
                for hh in range(2):
                    po = hh * dk
                    S = sp.tile([P, T], F32, tag="S")
                    Ss.append(S)
                    c0 = 0
                    while c0 < Lk - 256:
                        w = min(512, Lk - 256 - c0)
                        nc.tensor.matmul(S[:, c0:c0 + w],
                                         qT[po:po + dk, et, qs],
                                         kT[po:po + dk, et, c0:c0 + w],
                                         start=True, stop=True)
                        c0 += w
                    nc.tensor.matmul(S[:, Lk - 256:Lk],
                                     qT[po:po + dk, et, qs],
                                     kT[po:po + dk, et, Lk - 256:Lk],
                                     start=True, stop=False)
                    nc.tensor.matmul(S[:, Lk - 256:Lk], ident_bf,
                                     maskme[:, j * 256:(j + 1) * 256],
                                     start=False, stop=True)
                    e_ = abig.tile([P, T], F32, tag="e", bufs=5)
                    nc.scalar.activation(out=e_[:, :Lk], in_=S[:, :Lk],
                                         func=AF.Exp,
                                         accum_out=Zp[:, hh:hh + 1])
                    nc.vector.memset(x_[:, hh, Lk - 1:Lk], 0.0)
                    nc.vector.tensor_tensor_scan(
                        out=x_[:, hh, Lk - 2::-1], data0=e_[:, Lk - 1:0:-1],
                        data1=e_[:, Lk - 1:0:-1], initial=0.0,
                        op0=ALU.add, op1=ALU.bypass)
                # --- pair: x2 = suffix * (i-k); sqrt via ln/exp (same table
                # set as Exp); gz = -|gamma| / sqrt(Z + zfix) ---
                x2 = bass.AP(tensor=x_.tensor, offset=x_.offset,
                             ap=[x_.ap[0], [T, 2], [1, Lk]])
                nc.vector.scalar_tensor_tensor(
                    out=x2, in0=x2, scalar=-1.0,
                    in1=negp2, op0=ALU.mult, op1=ALU.mult)
                nc.scalar.activation(out=x2, in_=x2, func=AF.Ln)
                nc.scalar.activation(out=x2, in_=x2, func=AF.Exp, scale=0.5)
                nc.scalar.activation(out=gz, in_=Zp, func=AF.Ln,
                                     bias=eps30_col)
                nc.scalar.activation(out=gz, in_=gz, func=AF.Exp, scale=-0.5)
                nc.vector.tensor_tensor(out=gz, in0=gz,
                                        in1=gneg[:, h0:h0 + 2], op=ALU.mult)
                # --- stage B per head: f, S2 = S*f (psum), e2, rowmax ---
                for hh in range(2):
                    S = Ss[hh]
                    f_ = abig.tile([P, T], F32, tag="f", bufs=3)
                    nc.scalar.activation(out=f_[:, :Lk], in_=x_[:, hh, :Lk],
                                         func=AF.Exp, scale=gz[:, hh:hh + 1])
                    nc.vector.scalar_tensor_tensor(
                        out=S[:, :Lk], in0=f_[:, :Lk], scalar=1.0,
                        in1=S[:, :Lk], op0=ALU.mult, op1=ALU.mult)
                    e2 = abig.tile([P, T], BF16, tag="e2", bufs=4)
                    e2s.append(e2)
                    nc.scalar.activation(out=e2[:, :Lk], in_=S[:, :Lk],
                                         func=AF.Exp,
                                         accum_out=Z2p[:, hh:hh + 1])
                    nc.vector.tensor_reduce(
                        out=m2p[:, hh:hh + 1], in_=e2[:, :Lk],
                        axis=mybir.AxisListType.X, op=ALU.max)
                # --- pair tiny: cc = min(5/Z2', 1/m2') ---
                nc.vector.tensor_scalar_add(Z2p, Z2p, 1e-30)
                nc.vector.tensor_scalar_add(m2p, m2p, 1e-30)
                cc = asm.tile([P, 2], F32, tag="cc")
                rz2 = asm.tile([P, 2], F32, tag="rz2")
                nc.vector.reciprocal(out=rz2, in_=Z2p)
                rm2 = asm.tile([P, 2], F32, tag="rm2")
                nc.vector.reciprocal(out=rm2, in_=m2p)
                nc.vector.scalar_tensor_tensor(out=cc, in0=rz2, scalar=5.0,
                                               in1=rm2, op0=ALU.mult,
                                               op1=ALU.min)
                # --- stage C per head: rescale, transpose, PV ---
                opv = tp_pool.tile([P, 4 * P], F32, tag="opv")
                for hh in range(2):
                    h = h0 + hh
                    e2 = e2s[hh]
                    p_ = asm.tile([P, T], BF16, tag="p")
                    nc.vector.tensor_scalar_mul(p_[:, :Lk], e2[:, :Lk],
                                                cc[:, hh:hh + 1])
                    pT = asm.tile([P, TT, P], BF16, tag="pT")
                    nc.sync.dma_start_transpose(out=pT[:, :nkt, :],
                                                in_=p_[:, :Lk])
                    for kt in range(nkt):
                        nc.tensor.matmul(opv[hh * dk:(hh + 1) * dk, 0:P],
                                         vb[:, kt, h * dk:(h + 1) * dk],
                                         pT[:, kt, :],
                                         start=(kt == 0), stop=(kt == nkt - 1))
                nc.scalar.activation(out=concT[:, et, qs], in_=opv[:, 0:P],
                                     func=AF.Copy)

            # ---- output projection + residual + layernorm for q-tile j ----
            xsb = otmp.tile([P, D], F32, tag="xsb")
            for fh in range(2):
                fs = slice(fh * 512, (fh + 1) * 512)
                ps = tp_pool.tile([P, 4 * P], F32, tag="opv")
                for et2 in range(ET):
                    nc.tensor.matmul(ps, concT[:, et2, qs],
                                     woT_sb[:, et2, fs],
                                     start=(et2 == 0), stop=False)
                nc.tensor.matmul(ps, ones1_bf, bo_bf[:, fs],
                                 start=False, stop=True)
                nc.vector.tensor_tensor(out=xsb[:, fs], in0=ps,
                                        in1=qnat[:, j, fs], op=ALU.add)
            stats = omini.tile([P, 2, 6], F32, tag="stats")
            for sg in range(2):
                nc.vector.bn_stats(out=stats[:, sg, :],
                                   in_=xsb[:, sg * 512:(sg + 1) * 512])
            mv = omini.tile([P, 2], F32, tag="mv")
            nc.vector.bn_aggr(out=mv, in_=stats)
            rstd = omini.tile([P, 1], F32, tag="rstd")
            nc.scalar.activation(out=rstd, in_=mv[:, 1:2], func=AF.Ln,
                                 bias=eps_col)
            nc.scalar.activation(out=rstd, in_=rstd, func=AF.Exp, scale=-0.5)
            nmr = omini.tile([P, 1], F32, tag="nmr")
            nc.vector.scalar_tensor_tensor(out=nmr, in0=mv[:, 0:1], scalar=-1.0,
                                           in1=rstd, op0=ALU.mult, op1=ALU.mult)
            ysb = otmp.tile([P, D], F32, tag="ysb")
            nc.scalar.activation(out=ysb, in_=xsb, func=AF.Identity,
                                 bias=nmr, scale=rstd)
            nc.vector.tensor_tensor(out=ysb, in0=ysb, in1=lng_bc, op=ALU.mult)
            nc.vector.tensor_tensor(out=ysb, in0=ysb, in1=lnb_bc, op=ALU.add)
            nc.sync.dma_start(out=io["y"][qs, :], in_=ysb)

    st.close()


# ------------------------------------------------------------------
# program build + host-side runner
# ------------------------------------------------------------------

def build_program():
    nc = bacc.Bacc("TRN2", target_bir_lowering=False, debug=False,
                   num_devices=8)
    io = {}

    def inp(name, shape, dt=F32):
        io[name] = nc.dram_tensor(name, shape, dt, kind="ExternalInput").ap()

    inp("wqT", [D, D], BF16)
    inp("wvT", [D, D], BF16)
    inp("woT", [D, D], BF16)
    inp("xkT", [D, T], BF16)
    inp("xvT", [D, T], BF16)
    inp("xqT", [D, TQ], BF16)
    inp("xq", [TQ, D])
    inp("bq", [D])
    inp("bv", [D], BF16)
    inp("bo", [D], BF16)
    inp("gam", [H])
    inp("lng", [D])
    inp("lnb", [D])
    inp("iota", [P, T])
    inp("gcol", [P, NQ])
    inp("maskme", [P, NQ * 2 * P], BF16)
    io["y"] = nc.dram_tensor("y", [TQ, D], F32, kind="ExternalOutput").ap()
    with tile.TileContext(nc) as tc:
        emit(tc, io)
    nc.compile()
    _unify_act_tables(nc)
    return nc


def _unify_act_tables(nc):
    """Retarget every ACT table load to natural_log_exp_and_others (which
    contains all functions this kernel uses: exp/ln/identity/copy/abs) and
    drop now-redundant consecutive loads.  The default chooser alternates
    exp_and_others <-> natural_log, costing ~2.7us per switch."""
    from concourse.hw_specs import get_activation_tables
    tables = get_activation_tables(nc.m.arch)
    names = list(tables.keys())
    target = names.index("natural_log_exp_and_others")
    allowed = tables["natural_log_exp_and_others"]
    used = set()
    for fn in nc.m.functions:
        for b in fn.blocks:
            for ins in b.instructions:
                if isinstance(ins, mybir.InstActivation):
                    used.add(ins.func)
    if not used <= allowed:
        return  # some function outside the combined set; keep default loads
    for fn in nc.m.functions:
        for b in fn.blocks:
            new = []
            cur = -1
            for ins in b.instructions:
                if (isinstance(ins, mybir.InstLoadActFuncSet)
                        and ins.sync_info is None):
                    ins.act_func_set_id = target
                    if cur == target:
                        continue
                    cur = target
                new.append(ins)
            b.instructions[:] = new


def make_in_maps(inputs):
    import ml_dtypes
    bf = ml_dtypes.bfloat16
    q = np.asarray(inputs["query"], np.float32)
    k = np.asarray(inputs["key"], np.float32)
    v = np.asarray(inputs["values"], np.float32)
    wqT = np.ascontiguousarray(np.asarray(inputs["Wq"], np.float32).T).astype(bf)
    wvT = np.ascontiguousarray(np.asarray(inputs["Wv"], np.float32).T).astype(bf)
    woT = np.ascontiguousarray(np.asarray(inputs["Wo"], np.float32).T).astype(bf)
    small = {
        "bq": np.ascontiguousarray(inputs["bq"], np.float32),
        "bv": np.ascontiguousarray(inputs["bv"], np.float32).astype(bf),
        "bo": np.ascontiguousarray(inputs["bo"], np.float32).astype(bf),
        "gam": np.ascontiguousarray(inputs["gammas"], np.float32),
        "lng": np.ascontiguousarray(inputs["ln_g"], np.float32),
        "lnb": np.ascontiguousarray(inputs["ln_b"], np.float32),
    }
    iota = (np.arange(T)[None, :] - np.arange(P)[:, None]).astype(np.float32)

    stripe_data = []
    for qtiles in (QT_A, QT_B):
        rows = np.concatenate([np.arange(g * P, (g + 1) * P) for g in qtiles])
        gcol = np.zeros((P, NQ), np.float32)
        maskme = np.zeros((P, NQ, 2, P), np.float32)
        for jj, gi in enumerate(qtiles):
            gcol[:, jj] = -float(gi * P)
            i_glob = gi * P + np.arange(P)[:, None]
            for tt in range(2):
                tpos = EXT[jj] - 2 + tt
                kk = tpos * P + np.arange(P)[None, :]
                maskme[:, jj, tt, :] = np.where(kk >= i_glob, NEG, 0.0)
        stripe_data.append(dict(
            rows=rows, gcol=gcol,
            maskme=maskme.reshape(P, NQ * 2 * P)))

    maps = []
    for c in range(8):
        sd = stripe_data[c // 4]
        b = c % 4
        rows = sd["rows"]
        m = dict(small)
        m["wqT"], m["wvT"], m["woT"] = wqT, wvT, woT
        m["xkT"] = np.ascontiguousarray(k[b].T).astype(bf)
        m["xvT"] = np.ascontiguousarray(v[b].T).astype(bf)
        m["xqT"] = np.ascontiguousarray(q[b].T[:, rows]).astype(bf)
        m["xq"] = np.ascontiguousarray(q[b][rows])
        m["iota"] = iota
        m["gcol"] = sd["gcol"]
        m["maskme"] = sd["maskme"].astype(bf)
        maps.append(m)
    return maps


class _Runner:
    def __init__(self):
        self.nc = build_program()
        self._fn = None

    def _make_fn(self, nc, devices):
        import jax
        from jax.sharding import Mesh, PartitionSpec
        from jax.experimental.shard_map import shard_map
        from concourse import bass2jax
        from concourse.bass2jax import _bass_exec_p, partition_id_tensor

        bass2jax.install_neuronx_cc_hook()
        partition_name = (nc.partition_id_tensor.name
                          if nc.partition_id_tensor else None)
        in_names, out_names, out_avals, zero_outs = [], [], [], []
        for alloc in nc.m.functions[0].allocations:
            if not isinstance(alloc, mybir.MemoryLocationSet):
                continue
            name = alloc.memorylocations[0].name
            if alloc.kind == "ExternalInput":
                if name != partition_name:
                    in_names.append(name)
            elif alloc.kind == "ExternalOutput":
                shape = tuple(alloc.tensor_shape)
                dtype = mybir.dt.np(alloc.dtype)
                out_names.append(name)
                out_avals.append(jax.core.ShapedArray(shape, dtype))
                zero_outs.append(np.zeros(shape, dtype))
        n_params = len(in_names)
        all_in = list(in_names) + list(out_names)
        if partition_name is not None:
            all_in.append(partition_name)

        def _body(*args):
            operands = list(args)
            if partition_name is not None:
                operands.append(partition_id_tensor())
            outs = _bass_exec_p.bind(
                *operands, out_avals=tuple(out_avals), in_names=tuple(all_in),
                out_names=tuple(out_names), lowering_input_output_aliases=(),
                sim_require_finite=True, sim_require_nnan=True, nc=nc)
            return tuple(outs)

        mesh = Mesh(np.asarray(devices), ("core",))
        n = n_params + len(out_names)
        fn = jax.jit(shard_map(_body, mesh=mesh,
                               in_specs=(PartitionSpec("core"),) * n,
                               out_specs=(PartitionSpec("core"),) * len(out_names),
                               check_rep=False),
                     keep_unused=True)
        return fn, in_names, out_names, zero_outs

    def fn(self):
        if self._fn is None:
            import jax
            self._fn = self._make_fn(self.nc, jax.devices()[:8])
        return self._fn

    def run(self, inputs):
        import jax
        fn, in_names, out_names, zero_outs = self.fn()
        maps = make_in_maps(inputs)
        args = [np.concatenate([np.asarray(m[nm]) for m in maps], axis=0)
                for nm in in_names]
        args += [np.zeros((8 * z.shape[0], *z.shape[1:]), z.dtype)
                 for z in zero_outs]
        outs = fn(*args)
        jax.block_until_ready(outs)
        y = np.asarray(outs[0]).reshape(8, TQ, D)
        out = np.empty((B, T, D), np.float32)
        for c in range(8):
            qtiles = (QT_A, QT_B)[c // 4]
            b = c % 4
            for jj, g in enumerate(qtiles):
                out[b, g * P:(g + 1) * P] = y[c, jj * P:(jj + 1) * P]
        return out


_runner = None


def kernel(**inputs) -> np.ndarray:
    global _runner
    if _runner is None:
        _runner = _Runner()
    return _runner.run(inputs)
